# revision 1
# baseline (speedup 1.0000x reference)
"""Trainium2 Bass kernel for nn_AttentionTSSA.

Math (per batch b, head h, channel d, position n; N = T*V = 1600):
  w  = Wqkv @ x_b                      # [C, N] channel-mixing matmul
  s  = w^2
  D  = cumsum_n(s) (+1e-12)            # per channel
  u  = s / D
  R_h = temp_h * sum_d u               # per head (+32*bias_h*temp_h via exp bias)
  Pi = softmax_h(R)                    # softmax over the 8 heads
  CP = cumsum_n(Pi) + 1e-8             # per head
  Z  = cumsum_n(s*Pi + Pi) + 1e-8      # = CP + cumsum(s*Pi)  per channel
  attn = CP / Z                        # = 1/(1+dots)
  y  = -w * Pi * attn                  # (minus folded into -Wproj)
  out = Wproj @ y + bproj

Layout: channels on partitions (2 blocks of 128), n on free dim.
Head-coupled [8,N] work is stacked across groups of 4 batches into
[32,N] tiles.  Head<->channel reshapes go through PE indicator matmuls.
Matmuls run in float32r (1 cyc/row); cumsums via tensor_tensor_scan;
reciprocals via reciprocal_approx_fast.

Sharding: pure data parallel over B: 64 batches -> 8 cores x 8 batches.
"""

import numpy as np

# ---------------------------------------------------------------- constants
B, C, T, V = 64, 256, 64, 25
H = 8
DH = C // H                # 32 dims per head
N = T * V                  # 1600
NCORES = 8
BPC = B // NCORES          # 8 batches per core
NB = C // 128              # 2 channel blocks
HPB = 4                    # heads per 128-block
GRP = 4                    # batches per softmax group
NGRP = BPC // GRP
NROW = GRP * H             # 32 stacked head-rows per group

_CACHE = {}


def _build(middle_fp16: bool, use_dmax: bool):
    """Trace + compile the bass kernel; returns nc."""
    import concourse.bass as bass
    import concourse.tile as tile
    from concourse import bacc, mybir

    dt = mybir.dt
    MD = dt.float16 if middle_fp16 else dt.float32
    WD = dt.float16  # w-store dtype (SBUF footprint)
    AF = mybir.ActivationFunctionType
    OP = mybir.AluOpType

    nc = bacc.Bacc("TRN2", target_bir_lowering=False, debug=False)
    R = dt.float32r

    x_d = nc.dram_tensor("x", [BPC, C, N], R, kind="ExternalInput").ap()
    wqkvT_d = nc.dram_tensor("wqkvT", [NB, 128, C], R, kind="ExternalInput").ap()
    wprojTn_d = nc.dram_tensor("wprojTn", [NB, 128, C], R, kind="ExternalInput").ap()
    rlhs_d = nc.dram_tensor("rlhs", [NGRP, NB, GRP, 128, NROW], R,
                            kind="ExternalInput").ap()
    bc16_d = nc.dram_tensor("bc16", [NGRP, NB, GRP, NROW, 128],
                            dt.float16 if middle_fp16 else R,
                            kind="ExternalInput").ap()
    bcf_d = nc.dram_tensor("bcf", [NGRP, NB, GRP, NROW, 128], R,
                           kind="ExternalInput").ap()
    sumexp_d = nc.dram_tensor("sumexp", [NROW, NROW], R,
                              kind="ExternalInput").ap()
    expbias_d = nc.dram_tensor("expbias", [NROW, NGRP], dt.float32,
                               kind="ExternalInput").ap()
    bproj_d = nc.dram_tensor("bproj2", [128, NB], dt.float32, kind="ExternalInput").ap()
    out_d = nc.dram_tensor("out", [BPC, C, N], dt.float32, kind="ExternalOutput").ap()

    def f32r(ap):
        return ap.bitcast(R)

    CH_B = [(0, 512), (512, 512), (1024, 512), (1536, 64)]

    with tile.TileContext(nc) as tc:
        with (
            tc.tile_pool(name="const", bufs=1) as pc,
            tc.tile_pool(name="wstore", bufs=4) as pwst,
            tc.tile_pool(name="stackp", bufs=1) as pstk,
            tc.tile_pool(name="stmp", bufs=2) as pstm,
            tc.tile_pool(name="sa", bufs=3) as pa,
            tc.tile_pool(name="sbB", bufs=5) as pB,
            tc.tile_pool(name="yts", bufs=2) as pyt,
            tc.tile_pool(name="xin", bufs=1) as px,
            tc.tile_pool(name="oout", bufs=1) as pout,
            tc.tile_pool(name="psM", bufs=4, space="PSUM") as ppM,
            tc.tile_pool(name="psR", bufs=1, space="PSUM") as ppR,
        ):
            # ---- constants -----------------------------------------
            wqkvT = pc.tile([128, NB, C], R, tag="wqkvT")
            nc.sync.dma_start(wqkvT, wqkvT_d.rearrange("k p c -> p k c"))
            wprojTn = pc.tile([128, NB, C], R, tag="wprojTn")
            nc.sync.dma_start(wprojTn, wprojTn_d.rearrange("k p c -> p k c"))
            rlhs = pc.tile([128, NGRP, NB, GRP, NROW], R, tag="rlhs")
            nc.sync.dma_start(rlhs, rlhs_d.rearrange("g j b p m -> p g j b m"))
            bc16 = pc.tile([NROW, NGRP, NB, GRP, 128], bc16_d.dtype, tag="bc16")
            nc.sync.dma_start(bc16, bc16_d.rearrange("g j b p m -> p g j b m"))
            bcf = pc.tile([NROW, NGRP, NB, GRP, 128], R, tag="bcf")
            nc.sync.dma_start(bcf, bcf_d.rearrange("g j b p m -> p g j b m"))
            sumexp = pc.tile([NROW, NROW], R, tag="sumexp")
            nc.sync.dma_start(sumexp, sumexp_d)
            expbias = pc.tile([NROW, NGRP], dt.float32, tag="expbias")
            nc.sync.dma_start(expbias, expbias_d)
            bproj = pc.tile([128, NB], dt.float32, tag="bproj")
            nc.sync.dma_start(bproj, bproj_d)

            wtiles = {}          # (g, bl) -> w tile
            Rps = {}
            stk = {}             # g -> (Pi16, rCP)

            def emit_A_batch(g, bl):
                bi = g * GRP + bl
                if bl == 0:
                    Rps[g] = ppR.tile([NROW, N], dt.float32, tag="Rp", name=f"Rp{g}")
                Rp = Rps[g]
                wb = pwst.tile([128, NB, N], WD, tag="w16")
                wtiles[(g, bl)] = wb
                xt = px.tile([128, NB, N], R, tag="xt")
                nc.sync.dma_start(
                    xt, x_d[bi].rearrange("(k p) n -> p k n", p=128))
                for j in range(NB):
                    s32 = pa.tile([128, N], dt.float32, tag="sA")
                    for coff, clen in CH_B:
                        pw = ppM.tile([128, 512], dt.float32, tag="mm")
                        for k in range(NB):
                            nc.tensor.matmul(
                                pw[:, :clen],
                                wqkvT[:, k, j * 128:(j + 1) * 128],
                                xt[:, k, coff:coff + clen],
                                start=(k == 0), stop=(k == NB - 1),
                            )
                        nc.scalar.copy(
                            wb[:, j, coff:coff + clen], pw[:, :clen])
                        nc.scalar.activation(
                            s32[:, coff:coff + clen], pw[:, :clen], AF.Square)
                    D32 = pa.tile([128, N], dt.float32, tag="sA")
                    nc.vector.tensor_tensor_scan(
                        D32, s32, s32, 1e-12, OP.add, OP.bypass)
                    if use_dmax:
                        nc.vector.tensor_scalar_max(D32, D32, 1e-12)
                    lD = pa.tile([128, N], dt.float32, tag="sA")
                    nc.scalar.activation(lD, D32, AF.Ln)
                    rD = pa.tile([128, N], dt.float32, tag="sA")
                    nc.scalar.activation(rD, lD, AF.Exp, scale=-1.0)
                    u32 = pa.tile([128, N], R, tag="sA")
                    nc.gpsimd.tensor_tensor(u32, s32, rD, OP.mult)
                    for soff, slen in CH_B:
                        nc.tensor.matmul(
                            Rp[:, soff:soff + slen],
                            rlhs[:, g, j, bl, :],
                            u32[:, soff:soff + slen],
                            start=(bl == 0 and j == 0),
                            stop=(bl == GRP - 1 and j == NB - 1),
                        )

            def emit_S(g):
                Rp = Rps.pop(g)
                E32 = pstm.tile([NROW, N], R, tag="st")
                nc.scalar.activation(E32, Rp, AF.Exp,
                                     bias=expbias[:, g:g + 1], scale=1.0)
                Sp = ppR.tile([NROW, N], dt.float32, tag="Rp")
                for soff, slen in CH_B:
                    nc.tensor.matmul(Sp[:, soff:soff + slen], sumexp,
                                     E32[:, soff:soff + slen],
                                     start=True, stop=True)
                rS = pstm.tile([NROW, N], dt.float32, tag="st")
                nc.vector.reciprocal_approx_fast(out=rS, in_=Sp)
                Pi16 = pstk.tile([NROW, N],
                                 dt.float16 if middle_fp16 else R, tag="Pi16")
                nc.vector.tensor_tensor(Pi16, E32, rS, OP.mult)
                CP32 = pstm.tile([NROW, N], dt.float32, tag="st")
                nc.vector.tensor_tensor_scan(
                    CP32, Pi16, Pi16, 1e-8, OP.add, OP.bypass)
                rCPf = pstm.tile([NROW, N], dt.float32, tag="st")
                nc.vector.reciprocal_approx_fast(out=rCPf, in_=CP32)
                rCP = pstk.tile([NROW, N], R, tag="rCP")
                nc.vector.tensor_copy(rCP, rCPf)
                stk[g] = (Pi16, rCP)

            def emit_bcast(g, u):
                Pi16, rCP = stk[g]
                bl, j = u
                PiB = pB.tile([128, N], dt.float32, tag="B")
                rCPb = pB.tile([128, N], dt.float32, tag="B")
                for coff, clen in CH_B:
                    pPi = ppM.tile([128, 512], dt.float32, tag="mm")
                    nc.tensor.matmul(
                        pPi[:, :clen], bc16[:, g, j, bl, :],
                        Pi16[:, coff:coff + clen], start=True, stop=True)
                    nc.scalar.copy(PiB[:, coff:coff + clen], pPi[:, :clen])
                    pCP = ppM.tile([128, 512], dt.float32, tag="mm")
                    nc.tensor.matmul(
                        pCP[:, :clen], bcf[:, g, j, bl, :],
                        rCP[:, coff:coff + clen], start=True, stop=True)
                    nc.scalar.copy(rCPb[:, coff:coff + clen], pCP[:, :clen])
                return PiB, rCPb

            def emit_wproj(g, bl, yts):
                bi = g * GRP + bl
                for jo in range(NB):
                    ot = pout.tile([128, N], dt.float32, tag="ot")
                    for coff, clen in CH_B:
                        po = ppM.tile([128, 512], dt.float32, tag="mm")
                        for k in range(NB):
                            nc.tensor.matmul(
                                po[:, :clen],
                                wprojTn[:, k, jo * 128:(jo + 1) * 128],
                                yts[k][:, coff:coff + clen],
                                start=(k == 0), stop=(k == NB - 1),
                            )
                        nc.scalar.activation(
                            ot[:, coff:coff + clen], po[:, :clen],
                            AF.Identity, bias=bproj[:, jo:jo + 1], scale=1.0)
                    nc.sync.dma_start(
                        out_d[bi, jo * 128:(jo + 1) * 128, :], ot)

            def emit_B_unit(g, u, tiles, yts):
                bl, j = u
                PiB, rCPb = tiles
                wv = wtiles[(g, bl)][:, j, :]
                wp = pB.tile([128, N], dt.float32, tag="B")
                nc.vector.tensor_tensor(wp, wv, PiB, OP.mult)
                q = pB.tile([128, N], dt.float32, tag="B")
                nc.vector.tensor_tensor(q, wv, wp, OP.mult)
                Z = pB.tile([128, N], dt.float32, tag="B")
                nc.vector.tensor_tensor_scan(Z, q, PiB, 1e-8, OP.add, OP.add)
                nc.vector.tensor_tensor(Z, Z, rCPb, OP.mult)
                at = pB.tile([128, N], dt.float32, tag="B")
                nc.vector.reciprocal_approx_fast(out=at, in_=Z)
                yt = pyt.tile([128, N], R, tag="yt")
                yts.append(yt)
                nc.gpsimd.tensor_tensor(yt, wp, at, OP.mult)

            units = [(bl, j) for bl in range(GRP) for j in range(NB)]
            for g in range(NGRP):
                for bl in range(GRP):
                    emit_A_batch(g, bl)
                emit_S(g)
                nxt = emit_bcast(g, units[0])
                yts = []
                for ui, u in enumerate(units):
                    tiles = nxt
                    emit_B_unit(g, u, tiles, yts)
                    if ui + 1 < len(units):
                        nxt = emit_bcast(g, units[ui + 1])
                    bl, j = u
                    if j == NB - 1:
                        emit_wproj(g, bl, yts)
                        yts = []
                        del wtiles[(g, bl)]

    nc.compile()
    return nc


def _host_constants(Wqkv, temp, denom_bias, Wproj, bproj, middle_fp16: bool):
    f32 = np.float32
    wqkvT = np.ascontiguousarray(Wqkv.T.reshape(NB, 128, C)).astype(f32)
    wprojTn = np.ascontiguousarray((-Wproj.T).reshape(NB, 128, C)).astype(f32)

    temp = temp.reshape(H)
    denom_bias = denom_bias.reshape(H)
    rlhs = np.zeros((NGRP, NB, GRP, 128, NROW), f32)
    bc = np.zeros((NGRP, NB, GRP, NROW, 128), f32)
    for g in range(NGRP):
        for j in range(NB):
            for bl in range(GRP):
                for p in range(128):
                    h = HPB * j + p // DH
                    rlhs[g, j, bl, p, H * bl + h] = temp[h]
                    bc[g, j, bl, H * bl + h, p] = 1.0
    sumexp = np.zeros((NROW, NROW), f32)
    for m in range(NROW):
        q = m // H
        sumexp[q * H:(q + 1) * H, m] = 1.0
    expbias = np.zeros((NROW, NGRP), f32)
    for g in range(NGRP):
        for bl in range(GRP):
            for h in range(H):
                expbias[H * bl + h, g] = DH * denom_bias[h] * temp[h]
    bproj2 = np.ascontiguousarray(bproj.reshape(NB, 128).T).astype(f32)

    return {
        "wqkvT": wqkvT,
        "wprojTn": wprojTn,
        "rlhs": rlhs,
        "bc16": bc.astype(np.float16) if middle_fp16 else bc,
        "bcf": bc,
        "sumexp": sumexp,
        "expbias": expbias,
        "bproj2": bproj2,
    }


def kernel(x, Wqkv, temp, denom_bias, Wproj, bproj, *,
           middle_fp16=False, _run=None):
    x = np.asarray(x, np.float32)
    Wqkv = np.asarray(Wqkv, np.float32)
    temp = np.asarray(temp, np.float32)
    denom_bias = np.asarray(denom_bias, np.float32)
    Wproj = np.asarray(Wproj, np.float32)
    bproj = np.asarray(bproj, np.float32)

    # guard: do all sequences have cumsum(s) >= ~1e-9 at n=0?
    w0 = x[:, :, 0, 0] @ Wqkv.T
    use_dmax = bool((w0.astype(np.float64) ** 2).min() < 1e-9)

    key = (middle_fp16, use_dmax)
    if key not in _CACHE:
        _CACHE[key] = _build(middle_fp16, use_dmax)
    nc = _CACHE[key]

    consts = _host_constants(Wqkv, temp, denom_bias, Wproj, bproj, middle_fp16)
    xr = x.reshape(B, C, N)
    in_maps = []
    for core in range(NCORES):
        m = dict(consts)
        m["x"] = np.ascontiguousarray(xr[core * BPC:(core + 1) * BPC])
        in_maps.append(m)

    if _run is None:
        from concourse import bass_utils
        res = bass_utils.run_bass_kernel_spmd(nc, in_maps, list(range(NCORES)))
        outs = [r["out"] for r in res.results]
    else:
        outs = _run(nc, in_maps)

    out = np.concatenate(outs, axis=0).reshape(B, C, T, V)
    return out.astype(np.float32)



# revision 2
# speedup vs baseline: 1.2400x; 1.2400x over previous
"""Trainium2 Bass kernel for nn_AttentionTSSA — v2.

Math per (batch b, head h, channel c, position n), N = T*V = 1600:
  w   = Wqkv @ x_b                     # [C, N]
  s   = w^2
  D   = cumsum_n(s) + eD               # per channel
  u   = s / D
  R_r = sum_{c in head} u              # PE indicator matmul -> [NROW, N]
  E   = exp(temp_h * R + DH*db_h*temp_h)
  S   = sum_h E ; Pi = E / S           # softmax over heads
  CP  = cumsum_n(Pi) + 1e-8
  alpha = Pi * CP
  q   = s * Pi                         # PiB = Pi broadcast to channels
  Z   = cumsum_n(q + Pi) + eZ          # = F + CP
  m   = alpha / Z                      # = Pi * attn   (attn = CP/Z)
  y   = -w * m
  out = Wproj @ y + bproj              # minus folded into -Wproj

Engine split: PE matmuls (fp32r/f16 rhs, 1 cyc/col); scans on Pool
(gpsimd); element mults/divides on DVE in fp16 (2x mode); PSUM
evacuations on Act (f16 out); output evac split DVE/Pool, f16 to HBM.
eD = eZ = 6e-5 keeps all f16 intermediates finite (error analysis: only
positions with cumsum < 6e-5 are touched; contribution << 2e-2).

PSUM: wide ring [128,800] bufs=3 (6 banks) shared by Wqkv tiles, Pi/alpha
broadcasts and Wproj outputs (all PE-produced, in emission order) +
rs ring [32,512] bufs=2 (2 banks) for R/S chunks.

Sharding: data parallel over B: 64 batches -> 8 cores x 8 batches.
"""

import numpy as np

B, C, T, V = 64, 256, 64, 25
H = 8
DH = C // H                # 32
N = T * V                  # 1600
NCORES = 8
BPC = B // NCORES          # 8 batches per core
NB = C // 128              # 2 channel blocks
HPB = 128 // DH            # 4 heads per block
GRP = 4                    # batches per softmax group
NGRP = BPC // GRP          # 2
NROW = GRP * H             # 32 rows in head-stack
EPS = 6e-5                 # scan init for D and Z (f16-safe)
CHW = 800                  # wide psum tile columns
RSC = 512                  # rs ring chunk columns

_CACHE = {}


def _build():
    import concourse.bass as bass
    import concourse.tile as tile
    from concourse import bacc, mybir

    dt = mybir.dt
    AF = mybir.ActivationFunctionType
    OP = mybir.AluOpType
    F16 = dt.float16
    R = dt.float32r

    nc = bacc.Bacc("TRN2", target_bir_lowering=False, debug=False)

    x_d = nc.dram_tensor("x", [BPC, C, N], R, kind="ExternalInput").ap()
    wqkvT_d = nc.dram_tensor("wqkvT", [NB, 128, C], R, kind="ExternalInput").ap()
    wprojTn_d = nc.dram_tensor("wprojTn", [NB, 128, C], F16,
                               kind="ExternalInput").ap()
    rlhs_d = nc.dram_tensor("rlhs", [NB, 128, DH], F16,
                            kind="ExternalInput").ap()
    bc_d = nc.dram_tensor("bc", [128, NB, 128], F16,
                          kind="ExternalInput").ap()
    sumexp_d = nc.dram_tensor("sumexp", [128, 128], R,
                              kind="ExternalInput").ap()
    ebias_d = nc.dram_tensor("ebias", [128, 1], dt.float32,
                             kind="ExternalInput").ap()
    escale_d = nc.dram_tensor("escale", [128, 1], dt.float32,
                              kind="ExternalInput").ap()
    bproj_d = nc.dram_tensor("bproj2", [128, NB], dt.float32,
                             kind="ExternalInput").ap()
    out_d = nc.dram_tensor("out", [BPC, C, N], F16, kind="ExternalOutput").ap()
    alscr_d = nc.dram_tensor("alscr", [NGRP, 128, N], F16, kind="Internal").ap()

    # n-halves for wide tiles; 512/288 slices inside each half (>=256 for
    # fp32r full rate)
    HALVES = [(0, CHW), (CHW, N - CHW)]
    SLICES = [(0, 512), (512, 288)]
    RCH = [(0, 512), (512, 512), (1024, 512), (1536, 64)]

    with tile.TileContext(nc) as tc:
        with (
            tc.tile_pool(name="const", bufs=1) as pc,
            tc.tile_pool(name="xin", bufs=2) as px,
            tc.tile_pool(name="wst", bufs=2 * GRP + 1) as pw,   # wb16 ring
            tc.tile_pool(name="sst", bufs=2 * GRP + 1) as psq,  # s16 ring
            tc.tile_pool(name="ust", bufs=2 * GRP + 1) as pu,   # u16 ring
            tc.tile_pool(name="dtm", bufs=2) as pd,             # D16
            tc.tile_pool(name="hstk", bufs=1) as ph,            # head-space
            tc.tile_pool(name="bphase", bufs=2) as pb,          # B-phase tmp
            tc.tile_pool(name="ytile", bufs=2) as py,           # yt
            tc.tile_pool(name="otile", bufs=2) as po,           # out16
            tc.tile_pool(name="wide", bufs=4, space="PSUM") as pW,
        ):
            wqkvT = pc.tile([128, NB, C], R, tag="wqkvT")
            nc.sync.dma_start(wqkvT, wqkvT_d.rearrange("k p c -> p k c"))
            wprojTn = pc.tile([128, NB, C], F16, tag="wprojTn")
            nc.sync.dma_start(wprojTn, wprojTn_d.rearrange("k p c -> p k c"))
            rlhs = pc.tile([128, NB, DH], F16, tag="rlhs")
            nc.sync.dma_start(rlhs, rlhs_d.rearrange("j p m -> p j m"))
            bc = pc.tile([128, NB, 128], F16, tag="bc")
            nc.sync.dma_start(bc, bc_d)
            sumexp = pc.tile([128, 128], R, tag="sumexp")
            nc.sync.dma_start(sumexp, sumexp_d)
            ebias = pc.tile([128, 1], dt.float32, tag="ebias")
            nc.sync.dma_start(ebias, ebias_d)
            escale = pc.tile([128, 1], dt.float32, tag="escale")
            nc.sync.dma_start(escale, escale_d)
            bproj = pc.tile([128, NB], dt.float32, tag="bproj")
            nc.sync.dma_start(bproj, bproj_d)

            wtiles = {}   # (bi, j) -> wb16 [128, N]
            stiles = {}   # (bi, j) -> s16 [128, N]
            utiles = {}   # (bi, j) -> u16 [128, N]

            def emit_A(g, bl):
                bi = g * GRP + bl
                xt = px.tile([128, NB, N], R, tag="xt")
                nc.sync.dma_start(
                    xt, x_d[bi].rearrange("(k p) n -> p k n", p=128))
                for j in range(NB):
                    wb = pw.tile([128, N], F16, tag="wb")
                    wtiles[(bi, j)] = wb
                    Wps = [pW.tile([128, CHW], dt.float32, tag="wide",
                                   name=f"Wp{hi}")
                           for hi in range(len(HALVES))]
                    # k outer: one ldweights per k
                    for k in range(NB):
                        for hi, (hoff, hlen) in enumerate(HALVES):
                            for soff, slen in SLICES:
                                nc.tensor.matmul(
                                    Wps[hi][:, soff:soff + slen],
                                    wqkvT[:, k, j * 128:(j + 1) * 128],
                                    xt[:, k, hoff + soff:hoff + soff + slen],
                                    start=(k == 0), stop=(k == NB - 1),
                                )
                    for hi, (hoff, hlen) in enumerate(HALVES):
                        nc.scalar.copy(wb[:, hoff:hoff + hlen],
                                       Wps[hi][:, :hlen])
                    s16 = psq.tile([128, N], F16, tag="s16")
                    stiles[(bi, j)] = s16
                    nc.vector.tensor_tensor(s16, wb, wb, OP.mult)
                    D32 = pd.tile([128, N], dt.float32, tag="D32")
                    nc.vector.tensor_tensor_scan(
                        D32, s16, s16, EPS, OP.add, OP.bypass)
                    nc.vector.reciprocal_approx_fast(out=D32, in_=D32)
                    rD16 = pd.tile([128, N], F16, tag="rD16")
                    nc.scalar.copy(rD16, D32)
                    u16 = pu.tile([128, N], F16, tag="u16")
                    utiles[(bi, j)] = u16
                    nc.vector.tensor_tensor(u16, s16, rD16, OP.mult)

            def emit_RS(g):
                """R-matmul, exp, head-softmax, CP, alpha on a padded
                128-row head stack: batch bl occupies rows 32*bl..32*bl+7;
                pad rows are written 0 by the indicator matmuls."""
                E32 = ph.tile([128, N], R, tag="E32")
                Pi16 = ph.tile([128, N], F16, tag="Pi16")
                for hoff, hlen in HALVES:
                    Rp = pW.tile([128, CHW], dt.float32, tag="wide")
                    for j in range(NB):
                        for bl in range(GRP):
                            bi = g * GRP + bl
                            for soff, slen in SLICES:
                                nc.tensor.matmul(
                                    Rp[DH * bl:DH * (bl + 1),
                                       soff:soff + slen],
                                    rlhs[:, j, :],
                                    utiles[(bi, j)][:,
                                        hoff + soff:hoff + soff + slen],
                                    start=(j == 0), stop=(j == NB - 1),
                                    tile_position=(0, DH * bl),
                                )
                    nc.scalar.activation(
                        E32[:, hoff:hoff + hlen], Rp[:, :hlen],
                        AF.Exp, bias=ebias[:, 0:1], scale=escale[:, 0:1])
                for bl in range(GRP):
                    for j in range(NB):
                        del utiles[(g * GRP + bl, j)]
                for hoff, hlen in HALVES:
                    Sp = pW.tile([128, CHW], dt.float32, tag="wide")
                    for soff, slen in SLICES:
                        nc.tensor.matmul(Sp[:, soff:soff + slen], sumexp,
                                         E32[:, hoff + soff:hoff + soff + slen],
                                         start=True, stop=True)
                    rS32 = ph.tile([128, CHW], dt.float32, tag="rS32",
                                   name=f"rS{hoff}")
                    nc.vector.reciprocal_approx_fast(
                        out=rS32, in_=Sp[:, :hlen])
                    nc.vector.tensor_tensor(
                        Pi16[:, hoff:hoff + hlen],
                        E32[:, hoff:hoff + hlen],
                        rS32.bitcast(R), OP.mult)
                CP16 = ph.tile([128, N], F16, tag="CP16")
                nc.vector.tensor_tensor_scan(
                    CP16, Pi16, Pi16, 1e-8, OP.add, OP.bypass)
                al16 = ph.tile([128, N], F16, tag="al16")
                nc.vector.tensor_tensor(al16, Pi16, CP16, OP.mult)
                nc.sync.dma_start(alscr_d[g], al16)
                return (Pi16,)

            def emit_B(g, bl, Pi16):
                bi = g * GRP + bl
                yts = []
                for j in range(NB):
                    PiB = pb.tile([128, N], F16, tag="PiB")
                    alB = pb.tile([128, N], F16, tag="alB")
                    for hoff, hlen in HALVES:
                        Bp = pW.tile([128, CHW], dt.float32, tag="wide")
                        for soff, slen in SLICES:
                            nc.tensor.matmul(
                                Bp[:, soff:soff + slen],
                                bc[DH * bl:DH * bl + H, j, :],
                                Pi16[DH * bl:DH * bl + H,
                                     hoff + soff:hoff + soff + slen],
                                start=True, stop=True,
                                tile_position=(DH * bl, 0))
                        nc.scalar.copy(PiB[:, hoff:hoff + hlen],
                                       Bp[:, :hlen])
                    for hsub in range(HPB):
                        r = DH * bl + HPB * j + hsub
                        nc.sync.dma_start(
                            alB[DH * hsub:DH * (hsub + 1), :],
                            alscr_d[g, r:r + 1, :].partition_broadcast(DH))
                    s16 = stiles.pop((bi, j))
                    wb = wtiles.pop((bi, j))
                    q16 = pb.tile([128, N], F16, tag="q16")
                    nc.gpsimd.tensor_tensor(q16, s16, PiB, OP.mult)
                    Z32 = pb.tile([128, N], dt.float32, tag="Z32")
                    nc.vector.tensor_tensor_scan(
                        Z32, q16, PiB, EPS, OP.add, OP.add)
                    nc.vector.reciprocal_approx_fast(out=Z32, in_=Z32)
                    rZ16 = pb.tile([128, N], F16, tag="rZ16")
                    nc.scalar.copy(rZ16, Z32)
                    m16 = pb.tile([128, N], F16, tag="m16")
                    nc.gpsimd.tensor_tensor(m16, alB, rZ16, OP.mult)
                    yt = py.tile([128, N], F16, tag="yt")
                    yts.append(yt)
                    nc.gpsimd.tensor_tensor(yt, wb, m16, OP.mult)
                # wproj + bias, evac f16, DMA out
                for jo in range(NB):
                    ot = po.tile([128, N], F16, tag="ot")
                    Ops = [pW.tile([128, CHW], dt.float32, tag="wide",
                                   name=f"Op{hi}")
                           for hi in range(len(HALVES))]
                    for k in range(NB):
                        for hi, (hoff, hlen) in enumerate(HALVES):
                            for soff, slen in SLICES:
                                nc.tensor.matmul(
                                    Ops[hi][:, soff:soff + slen],
                                    wprojTn[:, k, jo * 128:(jo + 1) * 128],
                                    yts[k][:, hoff + soff:hoff + soff + slen],
                                    start=(k == 0), stop=(k == NB - 1),
                                )
                    for hi, (hoff, hlen) in enumerate(HALVES):
                        nc.scalar.activation(
                            ot[:, hoff:hoff + hlen], Ops[hi][:, :hlen],
                            AF.Identity, bias=bproj[:, jo:jo + 1],
                            scale=1.0)
                    nc.sync.dma_start(
                        out_d[bi, jo * 128:(jo + 1) * 128, :], ot)

            # Interleave: A(g+1) batches between B(g) batches so PE/psum-ring
            # work from both phases pipelines.
            for bl in range(GRP):
                emit_A(0, bl)
            stk = emit_RS(0)
            for g in range(NGRP):
                nxt_g = g + 1
                for bl in range(GRP):
                    emit_B(g, bl, *stk)
                    if nxt_g < NGRP:
                        emit_A(nxt_g, bl)
                if nxt_g < NGRP:
                    stk = emit_RS(nxt_g)

    nc.compile()
    return nc


def _host_constants(Wqkv, temp, denom_bias, Wproj, bproj):
    f32 = np.float32
    f16 = np.float16
    wqkvT = np.ascontiguousarray(Wqkv.T.reshape(NB, 128, C)).astype(f32)
    wprojTn = np.ascontiguousarray((-Wproj.T).reshape(NB, 128, C)).astype(f16)

    temp = temp.reshape(H).astype(f32)
    denom_bias = denom_bias.reshape(H).astype(f32)
    rlhs = np.zeros((NB, 128, DH), f16)
    bc = np.zeros((128, NB, 128), f16)
    for j in range(NB):
        for p in range(128):
            h = HPB * j + p // DH          # global head 0..7
            rlhs[j, p, h] = 1.0            # row h within the 32-row block
            for q in range(GRP):
                bc[DH * q + h, j, p] = 1.0
    sumexp = np.zeros((128, 128), f32)
    for q in range(GRP):
        r0 = DH * q
        sumexp[r0:r0 + H, r0:r0 + H] = 1.0
    ebias = np.zeros((128, 1), f32)
    escale = np.zeros((128, 1), f32)
    for q in range(GRP):
        for h in range(H):
            ebias[DH * q + h, 0] = DH * denom_bias[h] * temp[h]
            escale[DH * q + h, 0] = temp[h]
    bproj2 = np.ascontiguousarray(bproj.reshape(NB, 128).T).astype(f32)

    return {
        "wqkvT": wqkvT,
        "wprojTn": wprojTn,
        "rlhs": rlhs,
        "bc": bc,
        "sumexp": sumexp,
        "ebias": ebias,
        "escale": escale,
        "bproj2": bproj2,
    }


def kernel(x, Wqkv, temp, denom_bias, Wproj, bproj, *, _run=None):
    x = np.asarray(x, np.float32)
    Wqkv = np.asarray(Wqkv, np.float32)
    temp = np.asarray(temp, np.float32)
    denom_bias = np.asarray(denom_bias, np.float32)
    Wproj = np.asarray(Wproj, np.float32)
    bproj = np.asarray(bproj, np.float32)

    if "nc" not in _CACHE:
        _CACHE["nc"] = _build()
    nc = _CACHE["nc"]

    consts = _host_constants(Wqkv, temp, denom_bias, Wproj, bproj)
    xr = x.reshape(B, C, N)
    in_maps = []
    for core in range(NCORES):
        m = dict(consts)
        m["x"] = np.ascontiguousarray(xr[core * BPC:(core + 1) * BPC])
        in_maps.append(m)

    if _run is None:
        from concourse import bass_utils
        res = bass_utils.run_bass_kernel_spmd(nc, in_maps, list(range(NCORES)))
        outs = [r["out"] for r in res.results]
    else:
        outs = _run(nc, in_maps)

    out = np.concatenate(outs, axis=0).reshape(B, C, T, V)
    return out.astype(np.float32)


# revision 3
# speedup vs baseline: 1.3241x; 1.0679x over previous
"""Trainium2 Bass kernel for nn_AttentionTSSA — v2.

Math per (batch b, head h, channel c, position n), N = T*V = 1600:
  w   = Wqkv @ x_b                     # [C, N]
  s   = w^2
  D   = cumsum_n(s) + eD               # per channel
  u   = s / D
  R_r = sum_{c in head} u              # PE indicator matmul -> [NROW, N]
  E   = exp(temp_h * R + DH*db_h*temp_h)
  S   = sum_h E ; Pi = E / S           # softmax over heads
  CP  = cumsum_n(Pi) + 1e-8
  alpha = Pi * CP
  q   = s * Pi                         # PiB = Pi broadcast to channels
  Z   = cumsum_n(q + Pi) + eZ          # = F + CP
  m   = alpha / Z                      # = Pi * attn   (attn = CP/Z)
  y   = -w * m
  out = Wproj @ y + bproj              # minus folded into -Wproj

Engine split: PE matmuls (fp32r/f16 rhs, 1 cyc/col); scans on Pool
(gpsimd); element mults/divides on DVE in fp16 (2x mode); PSUM
evacuations on Act (f16 out); output evac split DVE/Pool, f16 to HBM.
eD = eZ = 6e-5 keeps all f16 intermediates finite (error analysis: only
positions with cumsum < 6e-5 are touched; contribution << 2e-2).

PSUM: wide ring [128,800] bufs=3 (6 banks) shared by Wqkv tiles, Pi/alpha
broadcasts and Wproj outputs (all PE-produced, in emission order) +
rs ring [32,512] bufs=2 (2 banks) for R/S chunks.

Sharding: data parallel over B: 64 batches -> 8 cores x 8 batches.
"""

import numpy as np

B, C, T, V = 64, 256, 64, 25
H = 8
DH = C // H                # 32
N = T * V                  # 1600
NCORES = 8
BPC = B // NCORES          # 8 batches per core
NB = C // 128              # 2 channel blocks
HPB = 128 // DH            # 4 heads per block
GRP = 4                    # batches per softmax group
NGRP = BPC // GRP          # 2
NROW = GRP * H             # 32 rows in head-stack
EPS = 6e-5                 # scan init for D and Z (f16-safe)
CHW = 800                  # wide psum tile columns
RSC = 512                  # rs ring chunk columns

_CACHE = {}


def _build():
    import concourse.bass as bass
    import concourse.tile as tile
    from concourse import bacc, mybir

    dt = mybir.dt
    AF = mybir.ActivationFunctionType
    OP = mybir.AluOpType
    F16 = dt.float16
    R = dt.float32r

    nc = bacc.Bacc("TRN2", target_bir_lowering=False, debug=False)

    x_d = nc.dram_tensor("x", [BPC, C, N], R, kind="ExternalInput").ap()
    wqkvT_d = nc.dram_tensor("wqkvT", [NB, 128, C], R, kind="ExternalInput").ap()
    wprojTn_d = nc.dram_tensor("wprojTn", [NB, 128, C], F16,
                               kind="ExternalInput").ap()
    rlhs_d = nc.dram_tensor("rlhs", [NB, 128, DH], F16,
                            kind="ExternalInput").ap()
    bc_d = nc.dram_tensor("bc", [128, NB, 128], F16,
                          kind="ExternalInput").ap()
    sumexp_d = nc.dram_tensor("sumexp", [128, 128], R,
                              kind="ExternalInput").ap()
    ebias_d = nc.dram_tensor("ebias", [128, 1], dt.float32,
                             kind="ExternalInput").ap()
    escale_d = nc.dram_tensor("escale", [128, 1], dt.float32,
                              kind="ExternalInput").ap()
    bproj_d = nc.dram_tensor("bproj2", [128, NB], dt.float32,
                             kind="ExternalInput").ap()
    out_d = nc.dram_tensor("out", [BPC, C, N], F16, kind="ExternalOutput").ap()
    alscr_d = nc.dram_tensor("alscr", [NGRP, 128, N], F16, kind="Internal").ap()

    # n-halves for wide tiles; 512/288 slices inside each half (>=256 for
    # fp32r full rate)
    HALVES = [(0, CHW), (CHW, N - CHW)]
    SLICES = [(0, 512), (512, 288)]
    RCH = [(0, 512), (512, 512), (1024, 512), (1536, 64)]

    with tile.TileContext(nc) as tc:
        with (
            tc.tile_pool(name="const", bufs=1) as pc,
            tc.tile_pool(name="xin", bufs=2) as px,
            tc.tile_pool(name="wst", bufs=2 * GRP + 1) as pw,   # wb16 ring
            tc.tile_pool(name="sst", bufs=2 * GRP + 1) as psq,  # s16 ring
            tc.tile_pool(name="ust", bufs=2 * GRP + 1) as pu,   # u16 ring
            tc.tile_pool(name="dtm", bufs=2) as pd,             # D16
            tc.tile_pool(name="hstk", bufs=1) as ph,            # head-space
            tc.tile_pool(name="bphase", bufs=2) as pb,          # B-phase tmp
            tc.tile_pool(name="ytile", bufs=2) as py,           # yt
            tc.tile_pool(name="otile", bufs=2) as po,           # out16
            tc.tile_pool(name="wide", bufs=4, space="PSUM") as pW,
        ):
            wqkvT = pc.tile([128, NB, C], R, tag="wqkvT")
            nc.sync.dma_start(wqkvT, wqkvT_d.rearrange("k p c -> p k c"))
            wprojTn = pc.tile([128, NB, C], F16, tag="wprojTn")
            nc.sync.dma_start(wprojTn, wprojTn_d.rearrange("k p c -> p k c"))
            rlhs = pc.tile([128, NB, DH], F16, tag="rlhs")
            nc.sync.dma_start(rlhs, rlhs_d.rearrange("j p m -> p j m"))
            bc = pc.tile([128, NB, 128], F16, tag="bc")
            nc.sync.dma_start(bc, bc_d)
            sumexp = pc.tile([128, 128], R, tag="sumexp")
            nc.sync.dma_start(sumexp, sumexp_d)
            ebias = pc.tile([128, 1], dt.float32, tag="ebias")
            nc.sync.dma_start(ebias, ebias_d)
            escale = pc.tile([128, 1], dt.float32, tag="escale")
            nc.sync.dma_start(escale, escale_d)
            bproj = pc.tile([128, NB], dt.float32, tag="bproj")
            nc.sync.dma_start(bproj, bproj_d)

            wtiles = {}   # (bi, j) -> wb16 [128, N]
            stiles = {}   # (bi, j) -> s16 [128, N]
            utiles = {}   # (bi, j) -> u16 [128, N]

            def emit_A(g, bl):
                bi = g * GRP + bl
                xt = px.tile([128, NB, N], R, tag="xt")
                nc.sync.dma_start(
                    xt, x_d[bi].rearrange("(k p) n -> p k n", p=128))
                for j in range(NB):
                    wb = pw.tile([128, N], F16, tag="wb")
                    wtiles[(bi, j)] = wb
                    Wps = [pW.tile([128, CHW], dt.float32, tag="wide",
                                   name=f"Wp{hi}")
                           for hi in range(len(HALVES))]
                    # k outer: one ldweights per k
                    for k in range(NB):
                        for hi, (hoff, hlen) in enumerate(HALVES):
                            for soff, slen in SLICES:
                                nc.tensor.matmul(
                                    Wps[hi][:, soff:soff + slen],
                                    wqkvT[:, k, j * 128:(j + 1) * 128],
                                    xt[:, k, hoff + soff:hoff + soff + slen],
                                    start=(k == 0), stop=(k == NB - 1),
                                )
                    for hi, (hoff, hlen) in enumerate(HALVES):
                        nc.scalar.copy(wb[:, hoff:hoff + hlen],
                                       Wps[hi][:, :hlen])
                    s16 = psq.tile([128, N], F16, tag="s16")
                    stiles[(bi, j)] = s16
                    nc.vector.tensor_tensor(s16, wb, wb, OP.mult)
                    D32 = pd.tile([128, N], dt.float32, tag="D32")
                    nc.vector.tensor_tensor_scan(
                        D32, s16, s16, EPS, OP.add, OP.bypass)
                    nc.vector.reciprocal_approx_fast(out=D32, in_=D32)
                    rD16 = pd.tile([128, N], F16, tag="rD16")
                    nc.scalar.copy(rD16, D32)
                    u16 = pu.tile([128, N], F16, tag="u16")
                    utiles[(bi, j)] = u16
                    nc.gpsimd.tensor_tensor(u16, s16, rD16, OP.mult)

            def emit_RS(g):
                """R-matmul, exp, head-softmax, CP, alpha on a padded
                128-row head stack: batch bl occupies rows 32*bl..32*bl+7;
                pad rows are written 0 by the indicator matmuls."""
                E32 = ph.tile([128, N], R, tag="E32")
                Pi16 = ph.tile([128, N], F16, tag="Pi16")
                for hoff, hlen in HALVES:
                    Rp = pW.tile([128, CHW], dt.float32, tag="wide")
                    for j in range(NB):
                        for bl in range(GRP):
                            bi = g * GRP + bl
                            for soff, slen in SLICES:
                                nc.tensor.matmul(
                                    Rp[DH * bl:DH * (bl + 1),
                                       soff:soff + slen],
                                    rlhs[:, j, :],
                                    utiles[(bi, j)][:,
                                        hoff + soff:hoff + soff + slen],
                                    start=(j == 0), stop=(j == NB - 1),
                                    tile_position=(0, DH * bl),
                                )
                    nc.scalar.activation(
                        E32[:, hoff:hoff + hlen], Rp[:, :hlen],
                        AF.Exp, bias=ebias[:, 0:1], scale=escale[:, 0:1])
                for bl in range(GRP):
                    for j in range(NB):
                        del utiles[(g * GRP + bl, j)]
                for hoff, hlen in HALVES:
                    Sp = pW.tile([128, CHW], dt.float32, tag="wide")
                    for soff, slen in SLICES:
                        nc.tensor.matmul(Sp[:, soff:soff + slen], sumexp,
                                         E32[:, hoff + soff:hoff + soff + slen],
                                         start=True, stop=True)
                    rS32 = ph.tile([128, CHW], dt.float32, tag="rS32",
                                   name=f"rS{hoff}")
                    nc.vector.reciprocal_approx_fast(
                        out=rS32, in_=Sp[:, :hlen])
                    nc.vector.tensor_tensor(
                        Pi16[:, hoff:hoff + hlen],
                        E32[:, hoff:hoff + hlen],
                        rS32.bitcast(R), OP.mult)
                CP16 = ph.tile([128, N], F16, tag="CP16")
                nc.vector.tensor_tensor_scan(
                    CP16, Pi16, Pi16, 1e-8, OP.add, OP.bypass)
                al16 = ph.tile([128, N], F16, tag="al16")
                nc.vector.tensor_tensor(al16, Pi16, CP16, OP.mult)
                nc.sync.dma_start(alscr_d[g], al16)
                return (Pi16,)

            def emit_B(g, bl, Pi16):
                bi = g * GRP + bl
                yts = []
                for j in range(NB):
                    PiB = pb.tile([128, N], F16, tag="PiB")
                    alB = pb.tile([128, N], F16, tag="alB")
                    for hoff, hlen in HALVES:
                        Bp = pW.tile([128, CHW], dt.float32, tag="wide")
                        for soff, slen in SLICES:
                            nc.tensor.matmul(
                                Bp[:, soff:soff + slen],
                                bc[DH * bl:DH * bl + H, j, :],
                                Pi16[DH * bl:DH * bl + H,
                                     hoff + soff:hoff + soff + slen],
                                start=True, stop=True,
                                tile_position=(DH * bl, 0))
                        nc.scalar.copy(PiB[:, hoff:hoff + hlen],
                                       Bp[:, :hlen])
                    for hsub in range(HPB):
                        r = DH * bl + HPB * j + hsub
                        nc.sync.dma_start(
                            alB[DH * hsub:DH * (hsub + 1), :],
                            alscr_d[g, r:r + 1, :].partition_broadcast(DH))
                    s16 = stiles.pop((bi, j))
                    wb = wtiles.pop((bi, j))
                    q16 = pb.tile([128, N], F16, tag="q16")
                    nc.vector.tensor_tensor(q16, s16, PiB, OP.mult)
                    Z32 = pb.tile([128, N], dt.float32, tag="Z32")
                    nc.vector.tensor_tensor_scan(
                        Z32, q16, PiB, EPS, OP.add, OP.add)
                    nc.vector.reciprocal_approx_fast(out=Z32, in_=Z32)
                    rZ16 = pb.tile([128, N], F16, tag="rZ16")
                    nc.scalar.copy(rZ16, Z32)
                    m16 = pb.tile([128, N], F16, tag="m16")
                    nc.vector.tensor_tensor(m16, alB, rZ16, OP.mult)
                    yt = py.tile([128, N], F16, tag="yt")
                    yts.append(yt)
                    nc.vector.tensor_tensor(yt, wb, m16, OP.mult)
                # wproj + bias, evac f16, DMA out
                for jo in range(NB):
                    ot = po.tile([128, N], F16, tag="ot")
                    Ops = [pW.tile([128, CHW], dt.float32, tag="wide",
                                   name=f"Op{hi}")
                           for hi in range(len(HALVES))]
                    for k in range(NB):
                        for hi, (hoff, hlen) in enumerate(HALVES):
                            for soff, slen in SLICES:
                                nc.tensor.matmul(
                                    Ops[hi][:, soff:soff + slen],
                                    wprojTn[:, k, jo * 128:(jo + 1) * 128],
                                    yts[k][:, hoff + soff:hoff + soff + slen],
                                    start=(k == 0), stop=(k == NB - 1),
                                )
                    for hi, (hoff, hlen) in enumerate(HALVES):
                        nc.scalar.activation(
                            ot[:, hoff:hoff + hlen], Ops[hi][:, :hlen],
                            AF.Identity, bias=bproj[:, jo:jo + 1],
                            scale=1.0)
                    nc.sync.dma_start(
                        out_d[bi, jo * 128:(jo + 1) * 128, :], ot)

            # Interleave: A(g+1) batches between B(g) batches so PE/psum-ring
            # work from both phases pipelines.
            for bl in range(GRP):
                emit_A(0, bl)
            stk = emit_RS(0)
            for g in range(NGRP):
                nxt_g = g + 1
                for bl in range(GRP):
                    emit_B(g, bl, *stk)
                    if nxt_g < NGRP:
                        emit_A(nxt_g, bl)
                if nxt_g < NGRP:
                    stk = emit_RS(nxt_g)

    nc.compile()
    return nc


def _host_constants(Wqkv, temp, denom_bias, Wproj, bproj):
    f32 = np.float32
    f16 = np.float16
    wqkvT = np.ascontiguousarray(Wqkv.T.reshape(NB, 128, C)).astype(f32)
    wprojTn = np.ascontiguousarray((-Wproj.T).reshape(NB, 128, C)).astype(f16)

    temp = temp.reshape(H).astype(f32)
    denom_bias = denom_bias.reshape(H).astype(f32)
    rlhs = np.zeros((NB, 128, DH), f16)
    bc = np.zeros((128, NB, 128), f16)
    for j in range(NB):
        for p in range(128):
            h = HPB * j + p // DH          # global head 0..7
            rlhs[j, p, h] = 1.0            # row h within the 32-row block
            for q in range(GRP):
                bc[DH * q + h, j, p] = 1.0
    sumexp = np.zeros((128, 128), f32)
    for q in range(GRP):
        r0 = DH * q
        sumexp[r0:r0 + H, r0:r0 + H] = 1.0
    ebias = np.zeros((128, 1), f32)
    escale = np.zeros((128, 1), f32)
    for q in range(GRP):
        for h in range(H):
            ebias[DH * q + h, 0] = DH * denom_bias[h] * temp[h]
            escale[DH * q + h, 0] = temp[h]
    bproj2 = np.ascontiguousarray(bproj.reshape(NB, 128).T).astype(f32)

    return {
        "wqkvT": wqkvT,
        "wprojTn": wprojTn,
        "rlhs": rlhs,
        "bc": bc,
        "sumexp": sumexp,
        "ebias": ebias,
        "escale": escale,
        "bproj2": bproj2,
    }


def kernel(x, Wqkv, temp, denom_bias, Wproj, bproj, *, _run=None):
    x = np.asarray(x, np.float32)
    Wqkv = np.asarray(Wqkv, np.float32)
    temp = np.asarray(temp, np.float32)
    denom_bias = np.asarray(denom_bias, np.float32)
    Wproj = np.asarray(Wproj, np.float32)
    bproj = np.asarray(bproj, np.float32)

    if "nc" not in _CACHE:
        _CACHE["nc"] = _build()
    nc = _CACHE["nc"]

    consts = _host_constants(Wqkv, temp, denom_bias, Wproj, bproj)
    xr = x.reshape(B, C, N)
    in_maps = []
    for core in range(NCORES):
        m = dict(consts)
        m["x"] = np.ascontiguousarray(xr[core * BPC:(core + 1) * BPC])
        in_maps.append(m)

    if _run is None:
        from concourse import bass_utils
        res = bass_utils.run_bass_kernel_spmd(nc, in_maps, list(range(NCORES)))
        outs = [r["out"] for r in res.results]
    else:
        outs = _run(nc, in_maps)

    out = np.concatenate(outs, axis=0).reshape(B, C, T, V)
    return out.astype(np.float32)


# revision 4
# speedup vs baseline: 1.3597x; 1.0269x over previous
"""Trainium2 Bass kernel for nn_AttentionTSSA — v2.

Math per (batch b, head h, channel c, position n), N = T*V = 1600:
  w   = Wqkv @ x_b                     # [C, N]
  s   = w^2
  D   = cumsum_n(s) + eD               # per channel
  u   = s / D
  R_r = sum_{c in head} u              # PE indicator matmul -> [NROW, N]
  E   = exp(temp_h * R + DH*db_h*temp_h)
  S   = sum_h E ; Pi = E / S           # softmax over heads
  CP  = cumsum_n(Pi) + 1e-8
  alpha = Pi * CP
  q   = s * Pi                         # PiB = Pi broadcast to channels
  Z   = cumsum_n(q + Pi) + eZ          # = F + CP
  m   = alpha / Z                      # = Pi * attn   (attn = CP/Z)
  y   = -w * m
  out = Wproj @ y + bproj              # minus folded into -Wproj

Engine split: PE matmuls (fp32r/f16 rhs, 1 cyc/col); scans on Pool
(gpsimd); element mults/divides on DVE in fp16 (2x mode); PSUM
evacuations on Act (f16 out); output evac split DVE/Pool, f16 to HBM.
eD = eZ = 6e-5 keeps all f16 intermediates finite (error analysis: only
positions with cumsum < 6e-5 are touched; contribution << 2e-2).

PSUM: wide ring [128,800] bufs=3 (6 banks) shared by Wqkv tiles, Pi/alpha
broadcasts and Wproj outputs (all PE-produced, in emission order) +
rs ring [32,512] bufs=2 (2 banks) for R/S chunks.

Sharding: data parallel over B: 64 batches -> 8 cores x 8 batches.
"""

import numpy as np

B, C, T, V = 64, 256, 64, 25
H = 8
DH = C // H                # 32
N = T * V                  # 1600
NCORES = 8
BPC = B // NCORES          # 8 batches per core
NB = C // 128              # 2 channel blocks
HPB = 128 // DH            # 4 heads per block
GRP = 4                    # batches per softmax group
NGRP = BPC // GRP          # 2
NROW = GRP * H             # 32 rows in head-stack
EPS = 6e-5                 # scan init for D and Z (f16-safe)
CHW = 800                  # wide psum tile columns
RSC = 512                  # rs ring chunk columns

_CACHE = {}


def _build():
    import concourse.bass as bass
    import concourse.tile as tile
    from concourse import bacc, mybir

    dt = mybir.dt
    AF = mybir.ActivationFunctionType
    OP = mybir.AluOpType
    F16 = dt.float16
    R = dt.float32r

    nc = bacc.Bacc("TRN2", target_bir_lowering=False, debug=False)

    x_d = nc.dram_tensor("x", [BPC, C, N], R, kind="ExternalInput").ap()
    wqkvT_d = nc.dram_tensor("wqkvT", [NB, 128, C], R, kind="ExternalInput").ap()
    wprojTn_d = nc.dram_tensor("wprojTn", [NB, 128, C], F16,
                               kind="ExternalInput").ap()
    rlhs_d = nc.dram_tensor("rlhs", [NB, 128, DH], F16,
                            kind="ExternalInput").ap()
    bc_d = nc.dram_tensor("bc", [128, NB, 128], F16,
                          kind="ExternalInput").ap()
    sumexp_d = nc.dram_tensor("sumexp", [128, 128], R,
                              kind="ExternalInput").ap()
    ebias_d = nc.dram_tensor("ebias", [128, 1], dt.float32,
                             kind="ExternalInput").ap()
    escale_d = nc.dram_tensor("escale", [128, 1], dt.float32,
                              kind="ExternalInput").ap()
    bproj_d = nc.dram_tensor("bproj2", [128, NB], dt.float32,
                             kind="ExternalInput").ap()
    out_d = nc.dram_tensor("out", [BPC, C, N], F16, kind="ExternalOutput").ap()
    alscr_d = nc.dram_tensor("alscr", [NGRP, 128, N], F16, kind="Internal").ap()

    # n-halves for wide tiles; 512/288 slices inside each half (>=256 for
    # fp32r full rate)
    HALVES = [(0, CHW), (CHW, N - CHW)]
    SLICES = [(0, 512), (512, 288)]
    RCH = [(0, 512), (512, 512), (1024, 512), (1536, 64)]

    with tile.TileContext(nc) as tc:
        with (
            tc.tile_pool(name="const", bufs=1) as pc,
            tc.tile_pool(name="xin", bufs=2) as px,
            tc.tile_pool(name="wst", bufs=2 * GRP + 1) as pw,   # wb16 ring
            tc.tile_pool(name="sst", bufs=2 * GRP + 1) as psq,  # s16 ring
            tc.tile_pool(name="ust", bufs=2 * GRP + 1) as pu,   # u16 ring
            tc.tile_pool(name="dtm", bufs=2) as pd,             # D16
            tc.tile_pool(name="hstk", bufs=1) as ph,            # head-space
            tc.tile_pool(name="bphase", bufs=2) as pb,          # B-phase tmp
            tc.tile_pool(name="ytile", bufs=2) as py,           # yt
            tc.tile_pool(name="otile", bufs=2) as po,           # out16
            tc.tile_pool(name="wide", bufs=4, space="PSUM") as pW,
        ):
            wqkvT = pc.tile([128, NB, C], R, tag="wqkvT")
            nc.sync.dma_start(wqkvT, wqkvT_d.rearrange("k p c -> p k c"))
            wprojTn = pc.tile([128, NB, C], F16, tag="wprojTn")
            nc.sync.dma_start(wprojTn, wprojTn_d.rearrange("k p c -> p k c"))
            rlhs = pc.tile([128, NB, DH], F16, tag="rlhs")
            nc.sync.dma_start(rlhs, rlhs_d.rearrange("j p m -> p j m"))
            bc = pc.tile([128, NB, 128], F16, tag="bc")
            nc.sync.dma_start(bc, bc_d)
            sumexp = pc.tile([128, 128], R, tag="sumexp")
            nc.sync.dma_start(sumexp, sumexp_d)
            ebias = pc.tile([128, 1], dt.float32, tag="ebias")
            nc.sync.dma_start(ebias, ebias_d)
            escale = pc.tile([128, 1], dt.float32, tag="escale")
            nc.sync.dma_start(escale, escale_d)
            bproj = pc.tile([128, NB], dt.float32, tag="bproj")
            nc.sync.dma_start(bproj, bproj_d)

            wtiles = {}   # (bi, j) -> wb16 [128, N]
            stiles = {}   # (bi, j) -> s16 [128, N]
            utiles = {}   # (bi, j) -> u16 [128, N]

            def emit_A(g, bl):
                bi = g * GRP + bl
                xt = px.tile([128, NB, N], R, tag="xt")
                nc.sync.dma_start(
                    xt, x_d[bi].rearrange("(k p) n -> p k n", p=128))
                for j in range(NB):
                    wb = pw.tile([128, N], F16, tag="wb")
                    wtiles[(bi, j)] = wb
                    Wps = [pW.tile([128, CHW], dt.float32, tag="wide",
                                   name=f"Wp{hi}")
                           for hi in range(len(HALVES))]
                    # k outer: one ldweights per k
                    for k in range(NB):
                        for hi, (hoff, hlen) in enumerate(HALVES):
                            for soff, slen in SLICES:
                                nc.tensor.matmul(
                                    Wps[hi][:, soff:soff + slen],
                                    wqkvT[:, k, j * 128:(j + 1) * 128],
                                    xt[:, k, hoff + soff:hoff + soff + slen],
                                    start=(k == 0), stop=(k == NB - 1),
                                )
                    for hi, (hoff, hlen) in enumerate(HALVES):
                        nc.scalar.copy(wb[:, hoff:hoff + hlen],
                                       Wps[hi][:, :hlen])
                    s16 = psq.tile([128, N], F16, tag="s16")
                    stiles[(bi, j)] = s16
                    nc.vector.tensor_tensor(s16, wb, wb, OP.mult)
                    D32 = pd.tile([128, N], dt.float32, tag="D32")
                    nc.vector.tensor_tensor_scan(
                        D32, s16, s16, EPS, OP.add, OP.bypass)
                    nc.vector.reciprocal_approx_fast(out=D32, in_=D32)
                    rD16 = pd.tile([128, N], F16, tag="rD16")
                    nc.scalar.copy(rD16, D32)
                    u16 = pu.tile([128, N], F16, tag="u16")
                    utiles[(bi, j)] = u16
                    nc.gpsimd.tensor_tensor(u16, s16, rD16, OP.mult)

            def emit_RS(g):
                """R-matmul, exp, head-softmax, CP, alpha on a padded
                128-row head stack: batch bl occupies rows 32*bl..32*bl+7;
                pad rows are written 0 by the indicator matmuls."""
                E32 = ph.tile([128, N], R, tag="E32")
                Pi16 = ph.tile([128, N], F16, tag="Pi16")
                for hoff, hlen in HALVES:
                    Rp = pW.tile([128, CHW], dt.float32, tag="wide")
                    for j in range(NB):
                        for bl in range(GRP):
                            bi = g * GRP + bl
                            for soff, slen in SLICES:
                                nc.tensor.matmul(
                                    Rp[DH * bl:DH * (bl + 1),
                                       soff:soff + slen],
                                    rlhs[:, j, :],
                                    utiles[(bi, j)][:,
                                        hoff + soff:hoff + soff + slen],
                                    start=(j == 0), stop=(j == NB - 1),
                                    tile_position=(0, DH * bl),
                                )
                    nc.scalar.activation(
                        E32[:, hoff:hoff + hlen], Rp[:, :hlen],
                        AF.Exp, bias=ebias[:, 0:1], scale=escale[:, 0:1])
                for bl in range(GRP):
                    for j in range(NB):
                        del utiles[(g * GRP + bl, j)]
                for hoff, hlen in HALVES:
                    Sp = pW.tile([128, CHW], dt.float32, tag="wide")
                    for soff, slen in SLICES:
                        nc.tensor.matmul(Sp[:, soff:soff + slen], sumexp,
                                         E32[:, hoff + soff:hoff + soff + slen],
                                         start=True, stop=True)
                    rS32 = ph.tile([128, CHW], dt.float32, tag="rS32",
                                   name=f"rS{hoff}")
                    nc.vector.reciprocal_approx_fast(
                        out=rS32, in_=Sp[:, :hlen])
                    nc.vector.tensor_tensor(
                        Pi16[:, hoff:hoff + hlen],
                        E32[:, hoff:hoff + hlen],
                        rS32.bitcast(R), OP.mult)
                CP16 = ph.tile([128, N], F16, tag="CP16")
                nc.vector.tensor_tensor_scan(
                    CP16, Pi16, Pi16, 1e-8, OP.add, OP.bypass)
                al16 = ph.tile([128, N], F16, tag="al16")
                nc.vector.tensor_tensor(al16, Pi16, CP16, OP.mult)
                nc.sync.dma_start(alscr_d[g], al16)
                return (Pi16,)

            def emit_B(g, bl, Pi16):
                bi = g * GRP + bl
                yts = []
                for j in range(NB):
                    PiB = pb.tile([128, N], F16, tag="PiB")
                    alB = pb.tile([128, N], F16, tag="alB")
                    for hoff, hlen in HALVES:
                        Bp = pW.tile([128, CHW], dt.float32, tag="wide")
                        for soff, slen in SLICES:
                            nc.tensor.matmul(
                                Bp[:, soff:soff + slen],
                                bc[DH * bl:DH * bl + H, j, :],
                                Pi16[DH * bl:DH * bl + H,
                                     hoff + soff:hoff + soff + slen],
                                start=True, stop=True,
                                tile_position=(DH * bl, 0))
                        nc.scalar.copy(PiB[:, hoff:hoff + hlen],
                                       Bp[:, :hlen])
                    for hsub in range(HPB):
                        r = DH * bl + HPB * j + hsub
                        nc.sync.dma_start(
                            alB[DH * hsub:DH * (hsub + 1), :],
                            alscr_d[g, r:r + 1, :].partition_broadcast(DH))
                    s16 = stiles.pop((bi, j))
                    wb = wtiles.pop((bi, j))
                    q16 = pb.tile([128, N], F16, tag="q16")
                    nc.vector.tensor_tensor(q16, s16, PiB, OP.mult)
                    Z32 = pb.tile([128, N], dt.float32, tag="Z32")
                    nc.vector.tensor_tensor_scan(
                        Z32, q16, PiB, EPS, OP.add, OP.add)
                    nc.vector.reciprocal_approx_fast(out=Z32, in_=Z32)
                    rZ16 = pb.tile([128, N], F16, tag="rZ16")
                    nc.scalar.copy(rZ16, Z32)
                    m16 = pb.tile([128, N], F16, tag="m16")
                    nc.vector.tensor_tensor(m16, alB, rZ16, OP.mult)
                    yt = py.tile([128, N], F16, tag="yt")
                    yts.append(yt)
                    nc.vector.tensor_tensor(yt, wb, m16, OP.mult)
                # wproj + bias, evac f16, DMA out
                for jo in range(NB):
                    ot = po.tile([128, N], F16, tag="ot")
                    Ops = [pW.tile([128, CHW], dt.float32, tag="wide",
                                   name=f"Op{hi}")
                           for hi in range(len(HALVES))]
                    for k in range(NB):
                        for hi, (hoff, hlen) in enumerate(HALVES):
                            for soff, slen in SLICES:
                                nc.tensor.matmul(
                                    Ops[hi][:, soff:soff + slen],
                                    wprojTn[:, k, jo * 128:(jo + 1) * 128],
                                    yts[k][:, hoff + soff:hoff + soff + slen],
                                    start=(k == 0), stop=(k == NB - 1),
                                )
                    for hi, (hoff, hlen) in enumerate(HALVES):
                        nc.scalar.activation(
                            ot[:, hoff:hoff + hlen], Ops[hi][:, :hlen],
                            AF.Identity, bias=bproj[:, jo:jo + 1],
                            scale=1.0)
                    nc.sync.dma_start(
                        out_d[bi, jo * 128:(jo + 1) * 128, :], ot)

            # Interleave: A(g+1) batches between B(g) batches so PE/psum-ring
            # work from both phases pipelines.
            for bl in range(GRP):
                emit_A(0, bl)
            stk = emit_RS(0)
            for g in range(NGRP):
                nxt_g = g + 1
                for bl in range(GRP):
                    if nxt_g < NGRP:
                        emit_A(nxt_g, bl)
                    emit_B(g, bl, *stk)
                if nxt_g < NGRP:
                    stk = emit_RS(nxt_g)

    nc.compile()
    return nc


def _host_constants(Wqkv, temp, denom_bias, Wproj, bproj):
    f32 = np.float32
    f16 = np.float16
    wqkvT = np.ascontiguousarray(Wqkv.T.reshape(NB, 128, C)).astype(f32)
    wprojTn = np.ascontiguousarray((-Wproj.T).reshape(NB, 128, C)).astype(f16)

    temp = temp.reshape(H).astype(f32)
    denom_bias = denom_bias.reshape(H).astype(f32)
    rlhs = np.zeros((NB, 128, DH), f16)
    bc = np.zeros((128, NB, 128), f16)
    for j in range(NB):
        for p in range(128):
            h = HPB * j + p // DH          # global head 0..7
            rlhs[j, p, h] = 1.0            # row h within the 32-row block
            for q in range(GRP):
                bc[DH * q + h, j, p] = 1.0
    sumexp = np.zeros((128, 128), f32)
    for q in range(GRP):
        r0 = DH * q
        sumexp[r0:r0 + H, r0:r0 + H] = 1.0
    ebias = np.zeros((128, 1), f32)
    escale = np.zeros((128, 1), f32)
    for q in range(GRP):
        for h in range(H):
            ebias[DH * q + h, 0] = DH * denom_bias[h] * temp[h]
            escale[DH * q + h, 0] = temp[h]
    bproj2 = np.ascontiguousarray(bproj.reshape(NB, 128).T).astype(f32)

    return {
        "wqkvT": wqkvT,
        "wprojTn": wprojTn,
        "rlhs": rlhs,
        "bc": bc,
        "sumexp": sumexp,
        "ebias": ebias,
        "escale": escale,
        "bproj2": bproj2,
    }


def kernel(x, Wqkv, temp, denom_bias, Wproj, bproj, *, _run=None):
    x = np.asarray(x, np.float32)
    Wqkv = np.asarray(Wqkv, np.float32)
    temp = np.asarray(temp, np.float32)
    denom_bias = np.asarray(denom_bias, np.float32)
    Wproj = np.asarray(Wproj, np.float32)
    bproj = np.asarray(bproj, np.float32)

    if "nc" not in _CACHE:
        _CACHE["nc"] = _build()
    nc = _CACHE["nc"]

    consts = _host_constants(Wqkv, temp, denom_bias, Wproj, bproj)
    xr = x.reshape(B, C, N)
    in_maps = []
    for core in range(NCORES):
        m = dict(consts)
        m["x"] = np.ascontiguousarray(xr[core * BPC:(core + 1) * BPC])
        in_maps.append(m)

    if _run is None:
        from concourse import bass_utils
        res = bass_utils.run_bass_kernel_spmd(nc, in_maps, list(range(NCORES)))
        outs = [r["out"] for r in res.results]
    else:
        outs = _run(nc, in_maps)

    out = np.concatenate(outs, axis=0).reshape(B, C, T, V)
    return out.astype(np.float32)


# revision 5
# speedup vs baseline: 1.3898x; 1.0221x over previous
"""Trainium2 Bass kernel for nn_AttentionTSSA — v2.

Math per (batch b, head h, channel c, position n), N = T*V = 1600:
  w   = Wqkv @ x_b                     # [C, N]
  s   = w^2
  D   = cumsum_n(s) + eD               # per channel
  u   = s / D
  R_r = sum_{c in head} u              # PE indicator matmul -> [NROW, N]
  E   = exp(temp_h * R + DH*db_h*temp_h)
  S   = sum_h E ; Pi = E / S           # softmax over heads
  CP  = cumsum_n(Pi) + 1e-8
  alpha = Pi * CP
  q   = s * Pi                         # PiB = Pi broadcast to channels
  Z   = cumsum_n(q + Pi) + eZ          # = F + CP
  m   = alpha / Z                      # = Pi * attn   (attn = CP/Z)
  y   = -w * m
  out = Wproj @ y + bproj              # minus folded into -Wproj

Engine split: PE matmuls (fp32r/f16 rhs, 1 cyc/col); scans on Pool
(gpsimd); element mults/divides on DVE in fp16 (2x mode); PSUM
evacuations on Act (f16 out); output evac split DVE/Pool, f16 to HBM.
eD = eZ = 6e-5 keeps all f16 intermediates finite (error analysis: only
positions with cumsum < 6e-5 are touched; contribution << 2e-2).

PSUM: wide ring [128,800] bufs=3 (6 banks) shared by Wqkv tiles, Pi/alpha
broadcasts and Wproj outputs (all PE-produced, in emission order) +
rs ring [32,512] bufs=2 (2 banks) for R/S chunks.

Sharding: data parallel over B: 64 batches -> 8 cores x 8 batches.
"""

import numpy as np

B, C, T, V = 64, 256, 64, 25
H = 8
DH = C // H                # 32
N = T * V                  # 1600
NCORES = 8
BPC = B // NCORES          # 8 batches per core
NB = C // 128              # 2 channel blocks
HPB = 128 // DH            # 4 heads per block
GRP = 4                    # batches per softmax group
NGRP = BPC // GRP          # 2
NROW = GRP * H             # 32 rows in head-stack
EPS = 6e-5                 # scan init for D and Z (f16-safe)
CHW = 800                  # wide psum tile columns
RSC = 512                  # rs ring chunk columns

_CACHE = {}


def _build():
    import concourse.bass as bass
    import concourse.tile as tile
    from concourse import bacc, mybir

    dt = mybir.dt
    AF = mybir.ActivationFunctionType
    OP = mybir.AluOpType
    F16 = dt.float16
    R = dt.float32r

    nc = bacc.Bacc("TRN2", target_bir_lowering=False, debug=False)

    x_d = nc.dram_tensor("x", [BPC, C, N], R, kind="ExternalInput").ap()
    wqkvT_d = nc.dram_tensor("wqkvT", [NB, 128, C], R, kind="ExternalInput").ap()
    wprojTn_d = nc.dram_tensor("wprojTn", [NB, 128, C], F16,
                               kind="ExternalInput").ap()
    rlhs_d = nc.dram_tensor("rlhs", [NB, 128, DH], F16,
                            kind="ExternalInput").ap()
    bc_d = nc.dram_tensor("bc", [128, NB, 128], F16,
                          kind="ExternalInput").ap()
    sumexp_d = nc.dram_tensor("sumexp", [128, 128], R,
                              kind="ExternalInput").ap()
    ebias_d = nc.dram_tensor("ebias", [128, 1], dt.float32,
                             kind="ExternalInput").ap()
    escale_d = nc.dram_tensor("escale", [128, 1], dt.float32,
                              kind="ExternalInput").ap()
    bproj_d = nc.dram_tensor("bproj2", [128, NB], dt.float32,
                             kind="ExternalInput").ap()
    out_d = nc.dram_tensor("out", [BPC, C, N], F16, kind="ExternalOutput").ap()
    alscr_d = nc.dram_tensor("alscr", [NGRP, 128, N], F16, kind="Internal").ap()

    # n-halves for wide tiles; 512/288 slices inside each half (>=256 for
    # fp32r full rate)
    HALVES = [(0, CHW), (CHW, N - CHW)]
    SLICES = [(0, 512), (512, 288)]
    RCH = [(0, 512), (512, 512), (1024, 512), (1536, 64)]

    with tile.TileContext(nc) as tc:
        with (
            tc.tile_pool(name="const", bufs=1) as pc,
            tc.tile_pool(name="xin", bufs=2) as px,
            tc.tile_pool(name="wst", bufs=2 * GRP + 1) as pw,   # wb16 ring
            tc.tile_pool(name="sst", bufs=2 * GRP + 1) as psq,  # s16 ring
            tc.tile_pool(name="ust", bufs=2 * GRP + 1) as pu,   # u16 ring
            tc.tile_pool(name="dtm", bufs=2) as pd,             # D16
            tc.tile_pool(name="hstk", bufs=1) as ph,            # head-space
            tc.tile_pool(name="bphase", bufs=2) as pb,          # B-phase tmp
            tc.tile_pool(name="ytile", bufs=2) as py,           # yt
            tc.tile_pool(name="otile", bufs=2) as po,           # out16
            tc.tile_pool(name="wide", bufs=4, space="PSUM") as pW,
        ):
            wqkvT = pc.tile([128, NB, C], R, tag="wqkvT")
            nc.sync.dma_start(wqkvT, wqkvT_d.rearrange("k p c -> p k c"))
            wprojTn = pc.tile([128, NB, C], F16, tag="wprojTn")
            nc.sync.dma_start(wprojTn, wprojTn_d.rearrange("k p c -> p k c"))
            rlhs = pc.tile([128, NB, DH], F16, tag="rlhs")
            nc.sync.dma_start(rlhs, rlhs_d.rearrange("j p m -> p j m"))
            bc = pc.tile([128, NB, 128], F16, tag="bc")
            nc.sync.dma_start(bc, bc_d)
            sumexp = pc.tile([128, 128], R, tag="sumexp")
            nc.sync.dma_start(sumexp, sumexp_d)
            ebias = pc.tile([128, 1], dt.float32, tag="ebias")
            nc.sync.dma_start(ebias, ebias_d)
            escale = pc.tile([128, 1], dt.float32, tag="escale")
            nc.sync.dma_start(escale, escale_d)
            bproj = pc.tile([128, NB], dt.float32, tag="bproj")
            nc.sync.dma_start(bproj, bproj_d)

            wtiles = {}   # (bi, j) -> wb16 [128, N]
            stiles = {}   # (bi, j) -> s16 [128, N]
            utiles = {}   # (bi, j) -> u16 [128, N]

            def emit_A(g, bl):
                bi = g * GRP + bl
                xt = px.tile([128, NB, N], R, tag="xt")
                for kk in range(NB):
                    for hoff, hlen in HALVES:
                        nc.sync.dma_start(
                            xt[:, kk, hoff:hoff + hlen],
                            x_d[bi, kk * 128:(kk + 1) * 128,
                                hoff:hoff + hlen])
                for j in range(NB):
                    wb = pw.tile([128, N], F16, tag="wb")
                    wtiles[(bi, j)] = wb
                    Wps = [pW.tile([128, CHW], dt.float32, tag="wide",
                                   name=f"Wp{hi}")
                           for hi in range(len(HALVES))]
                    # k outer: one ldweights per k
                    for k in range(NB):
                        for hi, (hoff, hlen) in enumerate(HALVES):
                            for soff, slen in SLICES:
                                nc.tensor.matmul(
                                    Wps[hi][:, soff:soff + slen],
                                    wqkvT[:, k, j * 128:(j + 1) * 128],
                                    xt[:, k, hoff + soff:hoff + soff + slen],
                                    start=(k == 0), stop=(k == NB - 1),
                                )
                    for hi, (hoff, hlen) in enumerate(HALVES):
                        nc.scalar.copy(wb[:, hoff:hoff + hlen],
                                       Wps[hi][:, :hlen])
                    s16 = psq.tile([128, N], F16, tag="s16")
                    stiles[(bi, j)] = s16
                    nc.vector.tensor_tensor(s16, wb, wb, OP.mult)
                    D32 = pd.tile([128, N], dt.float32, tag="D32")
                    nc.vector.tensor_tensor_scan(
                        D32, s16, s16, EPS, OP.add, OP.bypass)
                    nc.vector.reciprocal_approx_fast(out=D32, in_=D32)
                    rD16 = pd.tile([128, N], F16, tag="rD16")
                    nc.scalar.copy(rD16, D32)
                    u16 = pu.tile([128, N], F16, tag="u16")
                    utiles[(bi, j)] = u16
                    nc.gpsimd.tensor_tensor(u16, s16, rD16, OP.mult)

            def emit_RS(g):
                """R-matmul, exp, head-softmax, CP, alpha on a padded
                128-row head stack: batch bl occupies rows 32*bl..32*bl+7;
                pad rows are written 0 by the indicator matmuls."""
                E32 = ph.tile([128, N], R, tag="E32")
                Pi16 = ph.tile([128, N], F16, tag="Pi16")
                for hoff, hlen in HALVES:
                    Rp = pW.tile([128, CHW], dt.float32, tag="wide")
                    for j in range(NB):
                        for bl in range(GRP):
                            bi = g * GRP + bl
                            for soff, slen in SLICES:
                                nc.tensor.matmul(
                                    Rp[DH * bl:DH * (bl + 1),
                                       soff:soff + slen],
                                    rlhs[:, j, :],
                                    utiles[(bi, j)][:,
                                        hoff + soff:hoff + soff + slen],
                                    start=(j == 0), stop=(j == NB - 1),
                                    tile_position=(0, DH * bl),
                                )
                    nc.scalar.activation(
                        E32[:, hoff:hoff + hlen], Rp[:, :hlen],
                        AF.Exp, bias=ebias[:, 0:1], scale=escale[:, 0:1])
                for bl in range(GRP):
                    for j in range(NB):
                        del utiles[(g * GRP + bl, j)]
                for hoff, hlen in HALVES:
                    Sp = pW.tile([128, CHW], dt.float32, tag="wide")
                    for soff, slen in SLICES:
                        nc.tensor.matmul(Sp[:, soff:soff + slen], sumexp,
                                         E32[:, hoff + soff:hoff + soff + slen],
                                         start=True, stop=True)
                    rS32 = ph.tile([128, CHW], dt.float32, tag="rS32",
                                   name=f"rS{hoff}")
                    nc.vector.reciprocal_approx_fast(
                        out=rS32, in_=Sp[:, :hlen])
                    nc.vector.tensor_tensor(
                        Pi16[:, hoff:hoff + hlen],
                        E32[:, hoff:hoff + hlen],
                        rS32.bitcast(R), OP.mult)
                CP16 = ph.tile([128, N], F16, tag="CP16")
                nc.vector.tensor_tensor_scan(
                    CP16, Pi16, Pi16, 1e-8, OP.add, OP.bypass)
                al16 = ph.tile([128, N], F16, tag="al16")
                nc.vector.tensor_tensor(al16, Pi16, CP16, OP.mult)
                nc.sync.dma_start(alscr_d[g], al16)
                return (Pi16,)

            def emit_B(g, bl, Pi16):
                bi = g * GRP + bl
                yts = []
                for j in range(NB):
                    PiB = pb.tile([128, N], F16, tag="PiB")
                    alB = pb.tile([128, N], F16, tag="alB")
                    for hoff, hlen in HALVES:
                        Bp = pW.tile([128, CHW], dt.float32, tag="wide")
                        for soff, slen in SLICES:
                            nc.tensor.matmul(
                                Bp[:, soff:soff + slen],
                                bc[DH * bl:DH * bl + H, j, :],
                                Pi16[DH * bl:DH * bl + H,
                                     hoff + soff:hoff + soff + slen],
                                start=True, stop=True,
                                tile_position=(DH * bl, 0))
                        nc.scalar.copy(PiB[:, hoff:hoff + hlen],
                                       Bp[:, :hlen])
                    for hsub in range(HPB):
                        r = DH * bl + HPB * j + hsub
                        nc.sync.dma_start(
                            alB[DH * hsub:DH * (hsub + 1), :],
                            alscr_d[g, r:r + 1, :].partition_broadcast(DH))
                    s16 = stiles.pop((bi, j))
                    wb = wtiles.pop((bi, j))
                    q16 = pb.tile([128, N], F16, tag="q16")
                    nc.vector.tensor_tensor(q16, s16, PiB, OP.mult)
                    Z32 = pb.tile([128, N], dt.float32, tag="Z32")
                    nc.vector.tensor_tensor_scan(
                        Z32, q16, PiB, EPS, OP.add, OP.add)
                    nc.vector.reciprocal_approx_fast(out=Z32, in_=Z32)
                    rZ16 = pb.tile([128, N], F16, tag="rZ16")
                    nc.scalar.copy(rZ16, Z32)
                    m16 = pb.tile([128, N], F16, tag="m16")
                    nc.vector.tensor_tensor(m16, alB, rZ16, OP.mult)
                    yt = py.tile([128, N], F16, tag="yt")
                    yts.append(yt)
                    nc.vector.tensor_tensor(yt, wb, m16, OP.mult)
                # wproj + bias, evac f16, DMA out
                for jo in range(NB):
                    ot = po.tile([128, N], F16, tag="ot")
                    Ops = [pW.tile([128, CHW], dt.float32, tag="wide",
                                   name=f"Op{hi}")
                           for hi in range(len(HALVES))]
                    for k in range(NB):
                        for hi, (hoff, hlen) in enumerate(HALVES):
                            for soff, slen in SLICES:
                                nc.tensor.matmul(
                                    Ops[hi][:, soff:soff + slen],
                                    wprojTn[:, k, jo * 128:(jo + 1) * 128],
                                    yts[k][:, hoff + soff:hoff + soff + slen],
                                    start=(k == 0), stop=(k == NB - 1),
                                )
                    for hi, (hoff, hlen) in enumerate(HALVES):
                        nc.scalar.activation(
                            ot[:, hoff:hoff + hlen], Ops[hi][:, :hlen],
                            AF.Identity, bias=bproj[:, jo:jo + 1],
                            scale=1.0)
                        nc.sync.dma_start(
                            out_d[bi, jo * 128:(jo + 1) * 128,
                                  hoff:hoff + hlen],
                            ot[:, hoff:hoff + hlen])

            # Interleave: A(g+1) batches between B(g) batches so PE/psum-ring
            # work from both phases pipelines.
            for bl in range(GRP):
                emit_A(0, bl)
            stk = emit_RS(0)
            for g in range(NGRP):
                nxt_g = g + 1
                for bl in range(GRP):
                    if nxt_g < NGRP:
                        emit_A(nxt_g, bl)
                    emit_B(g, bl, *stk)
                if nxt_g < NGRP:
                    stk = emit_RS(nxt_g)

    nc.compile()
    return nc


def _host_constants(Wqkv, temp, denom_bias, Wproj, bproj):
    f32 = np.float32
    f16 = np.float16
    wqkvT = np.ascontiguousarray(Wqkv.T.reshape(NB, 128, C)).astype(f32)
    wprojTn = np.ascontiguousarray((-Wproj.T).reshape(NB, 128, C)).astype(f16)

    temp = temp.reshape(H).astype(f32)
    denom_bias = denom_bias.reshape(H).astype(f32)
    rlhs = np.zeros((NB, 128, DH), f16)
    bc = np.zeros((128, NB, 128), f16)
    for j in range(NB):
        for p in range(128):
            h = HPB * j + p // DH          # global head 0..7
            rlhs[j, p, h] = 1.0            # row h within the 32-row block
            for q in range(GRP):
                bc[DH * q + h, j, p] = 1.0
    sumexp = np.zeros((128, 128), f32)
    for q in range(GRP):
        r0 = DH * q
        sumexp[r0:r0 + H, r0:r0 + H] = 1.0
    ebias = np.zeros((128, 1), f32)
    escale = np.zeros((128, 1), f32)
    for q in range(GRP):
        for h in range(H):
            ebias[DH * q + h, 0] = DH * denom_bias[h] * temp[h]
            escale[DH * q + h, 0] = temp[h]
    bproj2 = np.ascontiguousarray(bproj.reshape(NB, 128).T).astype(f32)

    return {
        "wqkvT": wqkvT,
        "wprojTn": wprojTn,
        "rlhs": rlhs,
        "bc": bc,
        "sumexp": sumexp,
        "ebias": ebias,
        "escale": escale,
        "bproj2": bproj2,
    }


def kernel(x, Wqkv, temp, denom_bias, Wproj, bproj, *, _run=None):
    x = np.asarray(x, np.float32)
    Wqkv = np.asarray(Wqkv, np.float32)
    temp = np.asarray(temp, np.float32)
    denom_bias = np.asarray(denom_bias, np.float32)
    Wproj = np.asarray(Wproj, np.float32)
    bproj = np.asarray(bproj, np.float32)

    if "nc" not in _CACHE:
        _CACHE["nc"] = _build()
    nc = _CACHE["nc"]

    consts = _host_constants(Wqkv, temp, denom_bias, Wproj, bproj)
    xr = x.reshape(B, C, N)
    in_maps = []
    for core in range(NCORES):
        m = dict(consts)
        m["x"] = np.ascontiguousarray(xr[core * BPC:(core + 1) * BPC])
        in_maps.append(m)

    if _run is None:
        from concourse import bass_utils
        res = bass_utils.run_bass_kernel_spmd(nc, in_maps, list(range(NCORES)))
        outs = [r["out"] for r in res.results]
    else:
        outs = _run(nc, in_maps)

    out = np.concatenate(outs, axis=0).reshape(B, C, T, V)
    return out.astype(np.float32)


# revision 6
# speedup vs baseline: 1.4944x; 1.0753x over previous
"""Trainium2 Bass kernel for nn_AttentionTSSA — v2.

Math per (batch b, head h, channel c, position n), N = T*V = 1600:
  w   = Wqkv @ x_b                     # [C, N]
  s   = w^2
  D   = cumsum_n(s) + eD               # per channel
  u   = s / D
  R_r = sum_{c in head} u              # PE indicator matmul -> [NROW, N]
  E   = exp(temp_h * R + DH*db_h*temp_h)
  S   = sum_h E ; Pi = E / S           # softmax over heads
  CP  = cumsum_n(Pi) + 1e-8
  alpha = Pi * CP
  q   = s * Pi                         # PiB = Pi broadcast to channels
  Z   = cumsum_n(q + Pi) + eZ          # = F + CP
  m   = alpha / Z                      # = Pi * attn   (attn = CP/Z)
  y   = -w * m
  out = Wproj @ y + bproj              # minus folded into -Wproj

Engine split: PE matmuls (fp32r/f16 rhs, 1 cyc/col); scans on Pool
(gpsimd); element mults/divides on DVE in fp16 (2x mode); PSUM
evacuations on Act (f16 out); output evac split DVE/Pool, f16 to HBM.
eD = eZ = 6e-5 keeps all f16 intermediates finite (error analysis: only
positions with cumsum < 6e-5 are touched; contribution << 2e-2).

PSUM: wide ring [128,800] bufs=3 (6 banks) shared by Wqkv tiles, Pi/alpha
broadcasts and Wproj outputs (all PE-produced, in emission order) +
rs ring [32,512] bufs=2 (2 banks) for R/S chunks.

Sharding: data parallel over B: 64 batches -> 8 cores x 8 batches.
"""

import numpy as np

B, C, T, V = 64, 256, 64, 25
H = 8
DH = C // H                # 32
N = T * V                  # 1600
NCORES = 8
BPC = B // NCORES          # 8 batches per core
NB = C // 128              # 2 channel blocks
HPB = 128 // DH            # 4 heads per block
GRP = 4                    # batches per softmax group
NGRP = BPC // GRP          # 2
NROW = GRP * H             # 32 rows in head-stack
EPS = 6e-5                 # scan init for D and Z (f16-safe)
CHW = 800                  # wide psum tile columns
RSC = 512                  # rs ring chunk columns

_CACHE = {}


def _build():
    import concourse.bass as bass
    import concourse.tile as tile
    from concourse import bacc, mybir

    dt = mybir.dt
    AF = mybir.ActivationFunctionType
    OP = mybir.AluOpType
    F16 = dt.float16
    R = dt.float32r

    nc = bacc.Bacc("TRN2", target_bir_lowering=False, debug=False)

    x_d = nc.dram_tensor("x", [BPC, C, N], R, kind="ExternalInput").ap()
    wqkvT_d = nc.dram_tensor("wqkvT", [NB, 128, C], R, kind="ExternalInput").ap()
    wprojTn_d = nc.dram_tensor("wprojTn", [NB, 128, C], F16,
                               kind="ExternalInput").ap()
    rlhs_d = nc.dram_tensor("rlhs", [NB, 128, DH], F16,
                            kind="ExternalInput").ap()
    bc_d = nc.dram_tensor("bc", [128, NB, 128], F16,
                          kind="ExternalInput").ap()
    sumexp_d = nc.dram_tensor("sumexp", [128, 128], R,
                              kind="ExternalInput").ap()
    ebias_d = nc.dram_tensor("ebias", [128, 1], dt.float32,
                             kind="ExternalInput").ap()
    escale_d = nc.dram_tensor("escale", [128, 1], dt.float32,
                              kind="ExternalInput").ap()
    bproj_d = nc.dram_tensor("bproj2", [128, NB], dt.float32,
                             kind="ExternalInput").ap()
    out_d = nc.dram_tensor("out", [BPC, C, N], F16, kind="ExternalOutput").ap()
    alscr_d = nc.dram_tensor("alscr", [NGRP, 128, N], F16, kind="Internal").ap()

    # n-halves for wide tiles; 512/288 slices inside each half (>=256 for
    # fp32r full rate)
    HALVES = [(0, CHW), (CHW, N - CHW)]
    SLICES = [(0, 512), (512, 288)]
    RCH = [(0, 512), (512, 512), (1024, 512), (1536, 64)]

    with tile.TileContext(nc) as tc:
        with (
            tc.tile_pool(name="const", bufs=1) as pc,
            tc.tile_pool(name="xin", bufs=2) as px,
            tc.tile_pool(name="wst", bufs=2 * GRP + 1) as pw,   # wb16 ring
            tc.tile_pool(name="sst", bufs=2 * GRP + 1) as psq,  # s16 ring
            tc.tile_pool(name="ust", bufs=2 * GRP + 1) as pu,   # u16 ring
            tc.tile_pool(name="dtm", bufs=2) as pd,             # D16
            tc.tile_pool(name="hstk", bufs=1) as ph,            # head-space
            tc.tile_pool(name="bphase", bufs=2) as pb,          # B-phase tmp
            tc.tile_pool(name="ytile", bufs=2) as py,           # yt
            tc.tile_pool(name="otile", bufs=2) as po,           # out16
            tc.tile_pool(name="wide", bufs=4, space="PSUM") as pW,
        ):
            wqkvT = pc.tile([128, NB, C], R, tag="wqkvT")
            nc.sync.dma_start(wqkvT, wqkvT_d.rearrange("k p c -> p k c"))
            wprojTn = pc.tile([128, NB, C], F16, tag="wprojTn")
            nc.sync.dma_start(wprojTn, wprojTn_d.rearrange("k p c -> p k c"))
            rlhs = pc.tile([128, NB, DH], F16, tag="rlhs")
            nc.sync.dma_start(rlhs, rlhs_d.rearrange("j p m -> p j m"))
            bc = pc.tile([128, NB, 128], F16, tag="bc")
            nc.sync.dma_start(bc, bc_d)
            sumexp = pc.tile([128, 128], R, tag="sumexp")
            nc.sync.dma_start(sumexp, sumexp_d)
            ebias = pc.tile([128, 1], dt.float32, tag="ebias")
            nc.sync.dma_start(ebias, ebias_d)
            escale = pc.tile([128, 1], dt.float32, tag="escale")
            nc.sync.dma_start(escale, escale_d)
            bproj = pc.tile([128, NB], dt.float32, tag="bproj")
            nc.sync.dma_start(bproj, bproj_d)

            wtiles = {}   # (bi, j) -> wb16 [128, N]
            stiles = {}   # (bi, j) -> s16 [128, N]
            utiles = {}   # (bi, j) -> u16 [128, N]

            def emit_A(g, bl):
                bi = g * GRP + bl
                xt = px.tile([128, NB, N], R, tag="xt")
                for kk in range(NB):
                    for hoff, hlen in HALVES:
                        nc.sync.dma_start(
                            xt[:, kk, hoff:hoff + hlen],
                            x_d[bi, kk * 128:(kk + 1) * 128,
                                hoff:hoff + hlen])
                for j in range(NB):
                    wb = pw.tile([128, N], F16, tag="wb")
                    wtiles[(bi, j)] = wb
                    Wps = [pW.tile([128, CHW], dt.float32, tag="wide",
                                   name=f"Wp{hi}")
                           for hi in range(len(HALVES))]
                    # k outer: one ldweights per k
                    for k in range(NB):
                        for hi, (hoff, hlen) in enumerate(HALVES):
                            for soff, slen in SLICES:
                                nc.tensor.matmul(
                                    Wps[hi][:, soff:soff + slen],
                                    wqkvT[:, k, j * 128:(j + 1) * 128],
                                    xt[:, k, hoff + soff:hoff + soff + slen],
                                    start=(k == 0), stop=(k == NB - 1),
                                )
                    for hi, (hoff, hlen) in enumerate(HALVES):
                        nc.scalar.copy(wb[:, hoff:hoff + hlen],
                                       Wps[hi][:, :hlen])
                    s16 = psq.tile([128, N], F16, tag="s16")
                    stiles[(bi, j)] = s16
                    D32 = pd.tile([128, N], dt.float32, tag="D32")
                    for hi, (hoff, hlen) in enumerate(HALVES):
                        hs = slice(hoff, hoff + hlen)
                        nc.vector.tensor_tensor(s16[:, hs], wb[:, hs],
                                                wb[:, hs], OP.mult)
                        nc.vector.tensor_tensor_scan(
                            D32[:, hs], s16[:, hs], s16[:, hs],
                            EPS if hi == 0 else D32[:, hoff - 1:hoff],
                            OP.add, OP.bypass)
                    nc.vector.reciprocal_approx_fast(out=D32, in_=D32)
                    rD16 = pd.tile([128, N], F16, tag="rD16")
                    nc.scalar.copy(rD16, D32)
                    u16 = pu.tile([128, N], F16, tag="u16")
                    utiles[(bi, j)] = u16
                    nc.gpsimd.tensor_tensor(u16, s16, rD16, OP.mult)

            def emit_RS(g):
                """R-matmul, exp, head-softmax, CP, alpha on a padded
                128-row head stack: batch bl occupies rows 32*bl..32*bl+7;
                pad rows are written 0 by the indicator matmuls."""
                E32 = ph.tile([128, N], R, tag="E32")
                Pi16 = ph.tile([128, N], F16, tag="Pi16")
                for hoff, hlen in HALVES:
                    Rp = pW.tile([128, CHW], dt.float32, tag="wide")
                    for j in range(NB):
                        for bl in range(GRP):
                            bi = g * GRP + bl
                            for soff, slen in SLICES:
                                nc.tensor.matmul(
                                    Rp[DH * bl:DH * (bl + 1),
                                       soff:soff + slen],
                                    rlhs[:, j, :],
                                    utiles[(bi, j)][:,
                                        hoff + soff:hoff + soff + slen],
                                    start=(j == 0), stop=(j == NB - 1),
                                    tile_position=(0, DH * bl),
                                )
                    nc.scalar.activation(
                        E32[:, hoff:hoff + hlen], Rp[:, :hlen],
                        AF.Exp, bias=ebias[:, 0:1], scale=escale[:, 0:1])
                for bl in range(GRP):
                    for j in range(NB):
                        del utiles[(g * GRP + bl, j)]
                for hoff, hlen in HALVES:
                    Sp = pW.tile([128, CHW], dt.float32, tag="wide")
                    for soff, slen in SLICES:
                        nc.tensor.matmul(Sp[:, soff:soff + slen], sumexp,
                                         E32[:, hoff + soff:hoff + soff + slen],
                                         start=True, stop=True)
                    rS32 = ph.tile([128, CHW], dt.float32, tag="rS32",
                                   name=f"rS{hoff}")
                    nc.vector.reciprocal_approx_fast(
                        out=rS32, in_=Sp[:, :hlen])
                    nc.vector.tensor_tensor(
                        Pi16[:, hoff:hoff + hlen],
                        E32[:, hoff:hoff + hlen],
                        rS32.bitcast(R), OP.mult)
                CP16 = ph.tile([128, N], F16, tag="CP16")
                nc.vector.tensor_tensor_scan(
                    CP16, Pi16, Pi16, 1e-8, OP.add, OP.bypass)
                al16 = ph.tile([128, N], F16, tag="al16")
                nc.vector.tensor_tensor(al16, Pi16, CP16, OP.mult)
                nc.sync.dma_start(alscr_d[g], al16)
                return (Pi16,)

            def emit_B(g, bl, Pi16):
                bi = g * GRP + bl
                yts = []
                for j in range(NB):
                    PiB = pb.tile([128, N], F16, tag="PiB")
                    alB = pb.tile([128, N], F16, tag="alB")
                    for hoff, hlen in HALVES:
                        Bp = pW.tile([128, CHW], dt.float32, tag="wide")
                        for soff, slen in SLICES:
                            nc.tensor.matmul(
                                Bp[:, soff:soff + slen],
                                bc[DH * bl:DH * bl + H, j, :],
                                Pi16[DH * bl:DH * bl + H,
                                     hoff + soff:hoff + soff + slen],
                                start=True, stop=True,
                                tile_position=(DH * bl, 0))
                        nc.scalar.copy(PiB[:, hoff:hoff + hlen],
                                       Bp[:, :hlen])
                    for hsub in range(HPB):
                        r = DH * bl + HPB * j + hsub
                        nc.sync.dma_start(
                            alB[DH * hsub:DH * (hsub + 1), :],
                            alscr_d[g, r:r + 1, :].partition_broadcast(DH))
                    s16 = stiles.pop((bi, j))
                    wb = wtiles.pop((bi, j))
                    q16 = pb.tile([128, N], F16, tag="q16")
                    Z32 = pb.tile([128, N], dt.float32, tag="Z32")
                    for hi, (hoff, hlen) in enumerate(HALVES):
                        hs = slice(hoff, hoff + hlen)
                        nc.vector.tensor_tensor(q16[:, hs], s16[:, hs],
                                                PiB[:, hs], OP.mult)
                        nc.vector.tensor_tensor_scan(
                            Z32[:, hs], q16[:, hs], PiB[:, hs],
                            EPS if hi == 0 else Z32[:, hoff - 1:hoff],
                            OP.add, OP.add)
                    nc.vector.reciprocal_approx_fast(out=Z32, in_=Z32)
                    rZ16 = pb.tile([128, N], F16, tag="rZ16")
                    nc.scalar.copy(rZ16, Z32)
                    m16 = pb.tile([128, N], F16, tag="m16")
                    nc.vector.tensor_tensor(m16, alB, rZ16, OP.mult)
                    yt = py.tile([128, N], F16, tag="yt")
                    yts.append(yt)
                    nc.vector.tensor_tensor(yt, wb, m16, OP.mult)
                # wproj + bias, evac f16, DMA out
                for jo in range(NB):
                    ot = po.tile([128, N], F16, tag="ot")
                    Ops = [pW.tile([128, CHW], dt.float32, tag="wide",
                                   name=f"Op{hi}")
                           for hi in range(len(HALVES))]
                    for k in range(NB):
                        for hi, (hoff, hlen) in enumerate(HALVES):
                            for soff, slen in SLICES:
                                nc.tensor.matmul(
                                    Ops[hi][:, soff:soff + slen],
                                    wprojTn[:, k, jo * 128:(jo + 1) * 128],
                                    yts[k][:, hoff + soff:hoff + soff + slen],
                                    start=(k == 0), stop=(k == NB - 1),
                                )
                    for hi, (hoff, hlen) in enumerate(HALVES):
                        nc.scalar.activation(
                            ot[:, hoff:hoff + hlen], Ops[hi][:, :hlen],
                            AF.Identity, bias=bproj[:, jo:jo + 1],
                            scale=1.0)
                        nc.sync.dma_start(
                            out_d[bi, jo * 128:(jo + 1) * 128,
                                  hoff:hoff + hlen],
                            ot[:, hoff:hoff + hlen])

            # Interleave: A(g+1) batches between B(g) batches so PE/psum-ring
            # work from both phases pipelines.
            for bl in range(GRP):
                emit_A(0, bl)
            stk = emit_RS(0)
            for g in range(NGRP):
                nxt_g = g + 1
                for bl in range(GRP):
                    if nxt_g < NGRP:
                        emit_A(nxt_g, bl)
                    emit_B(g, bl, *stk)
                if nxt_g < NGRP:
                    stk = emit_RS(nxt_g)

    nc.compile()
    return nc


def _host_constants(Wqkv, temp, denom_bias, Wproj, bproj):
    f32 = np.float32
    f16 = np.float16
    wqkvT = np.ascontiguousarray(Wqkv.T.reshape(NB, 128, C)).astype(f32)
    wprojTn = np.ascontiguousarray((-Wproj.T).reshape(NB, 128, C)).astype(f16)

    temp = temp.reshape(H).astype(f32)
    denom_bias = denom_bias.reshape(H).astype(f32)
    rlhs = np.zeros((NB, 128, DH), f16)
    bc = np.zeros((128, NB, 128), f16)
    for j in range(NB):
        for p in range(128):
            h = HPB * j + p // DH          # global head 0..7
            rlhs[j, p, h] = 1.0            # row h within the 32-row block
            for q in range(GRP):
                bc[DH * q + h, j, p] = 1.0
    sumexp = np.zeros((128, 128), f32)
    for q in range(GRP):
        r0 = DH * q
        sumexp[r0:r0 + H, r0:r0 + H] = 1.0
    ebias = np.zeros((128, 1), f32)
    escale = np.zeros((128, 1), f32)
    for q in range(GRP):
        for h in range(H):
            ebias[DH * q + h, 0] = DH * denom_bias[h] * temp[h]
            escale[DH * q + h, 0] = temp[h]
    bproj2 = np.ascontiguousarray(bproj.reshape(NB, 128).T).astype(f32)

    return {
        "wqkvT": wqkvT,
        "wprojTn": wprojTn,
        "rlhs": rlhs,
        "bc": bc,
        "sumexp": sumexp,
        "ebias": ebias,
        "escale": escale,
        "bproj2": bproj2,
    }


def kernel(x, Wqkv, temp, denom_bias, Wproj, bproj, *, _run=None):
    x = np.asarray(x, np.float32)
    Wqkv = np.asarray(Wqkv, np.float32)
    temp = np.asarray(temp, np.float32)
    denom_bias = np.asarray(denom_bias, np.float32)
    Wproj = np.asarray(Wproj, np.float32)
    bproj = np.asarray(bproj, np.float32)

    if "nc" not in _CACHE:
        _CACHE["nc"] = _build()
    nc = _CACHE["nc"]

    consts = _host_constants(Wqkv, temp, denom_bias, Wproj, bproj)
    xr = x.reshape(B, C, N)
    in_maps = []
    for core in range(NCORES):
        m = dict(consts)
        m["x"] = np.ascontiguousarray(xr[core * BPC:(core + 1) * BPC])
        in_maps.append(m)

    if _run is None:
        from concourse import bass_utils
        res = bass_utils.run_bass_kernel_spmd(nc, in_maps, list(range(NCORES)))
        outs = [r["out"] for r in res.results]
    else:
        outs = _run(nc, in_maps)

    out = np.concatenate(outs, axis=0).reshape(B, C, T, V)
    return out.astype(np.float32)
